# revision 37
# baseline (speedup 1.0000x reference)
"""Multi-head causal attention (B=2, S=2048, D=1024, H=16) on 8 trn2 NeuronCores.

Sharding: 2 heads per core (tensor-parallel over the 16 heads). Each core
receives the full (host-pre-transposed, fp16) activations plus its own slice
of the projection weights, computes

    qhT/khT = (Wq_l @ x.T)      [128, B*S]   (head dim on partitions)
    vh      = x @ Wv_l.T        per 128-row chunk, stored [j, c] + ones column
    S_T     = kh @ qh.T / 8     per (b, h-pair), keys on partitions
    P       = exp(S_T + pad_bias) * causal_mask
    attT    = (v_aug.T @ P) -> rows 0:64 = att.T, row 64 = softmax denominator
    aT      = attT / denom      (denominator broadcast via DMA replication)
    y_part  = A_local @ Wo[:, mslice].T     [B*S, D] partial sums

The host sums the 8 partial outputs (standard row-parallel unshard) and
reshapes to [B, S, D] float32.

Schedule: the exp-bound attention loops (phase B) are fed with phase-A
projection work units so the tensor engine fills its exp-wait bubbles:
B(b=0) hosts A(ss2)+A(ss3); B(b=1) hosts the NEXT repetition's A(ss0)+A(ss1)
plus the output-projection (phase C) tiles. DMA queues: Sync streams x tiles
only; the divide round-trip and y writes ride the idle Pool (gpsimd) queue.
"""

import os
import sys

import numpy as np

try:
    import concourse.bass as bass
except ImportError:  # fallback if sitecustomize did not add the repo
    for _p in ("/opt/trn_rl_repo", "/root/.axon_site/_ro/trn_rl_repo"):
        if os.path.isdir(_p) and _p not in sys.path:
            sys.path.insert(0, _p)
    import concourse.bass as bass  # noqa: F401

import concourse.tile as tile
from concourse import bacc, mybir
from concourse.bass_utils import run_bass_kernel_spmd

B, S, D, H, DK = 2, 2048, 1024, 16, 64
BS = B * S                # 4096
N_CORES = 8
H_LOC = H // N_CORES      # 2 heads per core
M_LOC = H_LOC * DK        # 128 local concat dim
NJT = S // 128            # 16 key tiles per batch

F16 = mybir.dt.float16
F32 = mybir.dt.float32
FT = mybir.ActivationFunctionType

_CACHE = {}


def _build_nc(reps=1):
    """Build + compile the per-core Bass program (identical across cores).

    reps > 1 repeats the whole (idempotent) body back-to-back inside one
    NEFF — used by test.py to time one rep via the slope between NEFFs.
    """
    from contextlib import ExitStack

    nc = bacc.Bacc(
        "TRN2", target_bir_lowering=False, debug=False, enable_asserts=False
    )

    io = {}
    for nm in ("xqT", "xkT", "xvT"):
        io[nm] = nc.dram_tensor(nm, [D, BS], F16, kind="ExternalInput").ap()
    for nm in ("wq", "wk", "wv"):
        io[nm] = nc.dram_tensor(nm, [128, 8, 128], F16, kind="ExternalInput").ap()
    io["woT"] = nc.dram_tensor("woT", [128, D], F16, kind="ExternalInput").ap()
    io["pad_bias"] = nc.dram_tensor(
        "pad_bias", [128, B, NJT], F32, kind="ExternalInput"
    ).ap()
    # causal masks for the 4 diagonal offsets, duplicated along the head pair:
    # [128, 4, 2*512]
    io["cmask"] = nc.dram_tensor(
        "cmask", [128, 4, 2 * 512], F16, kind="ExternalInput"
    ).ap()
    io["y"] = nc.dram_tensor("y_partial", [BS, D], F16, kind="ExternalOutput").ap()
    # scratch for the softmax-denominator broadcast (DRAM allows 0-step APs)
    io["rscratch"] = nc.dram_tensor("rscratch", [B * 4, H_LOC * 512], F16).ap()

    with tile.TileContext(nc) as tc, ExitStack() as ctx:
        pools = {
            "const": ctx.enter_context(tc.tile_pool(name="const", bufs=1)),
            "xpool": ctx.enter_context(tc.tile_pool(name="xpool", bufs=36)),
            "ppool": ctx.enter_context(tc.tile_pool(name="ppool", bufs=6)),
            "mpool": ctx.enter_context(tc.tile_pool(name="mpool", bufs=2)),
            "ypool": ctx.enter_context(tc.tile_pool(name="ypool", bufs=3)),
            # PSUM budget (8 banks): ps 2x2 + po 1x2 + mm 2x1 = 8
            "psum_s": ctx.enter_context(
                tc.tile_pool(name="psum_s", bufs=2, space="PSUM")
            ),
            "psum_o": ctx.enter_context(
                tc.tile_pool(name="psum_o", bufs=1, space="PSUM")
            ),
            "psum_mm": ctx.enter_context(
                tc.tile_pool(name="psum_mm", bufs=2, space="PSUM")
            ),
        }
        _program(pools, tc, io, reps)
    nc.compile()
    return nc


def _program(pools, tc, io, reps):
    nc = tc.nc

    const = pools["const"]
    xpool = pools["xpool"]
    ppool = pools["ppool"]
    mpool = pools["mpool"]
    ypool = pools["ypool"]
    psum_s = pools["psum_s"]
    psum_o = pools["psum_o"]
    psum_mm = pools["psum_mm"]

    # ---- constants / persistent buffers (loaded once for all reps) ----
    w_sb = {}
    for nm in ("wq", "wk", "wv"):
        t = const.tile([128, 8, 128], F16, name=f"{nm}_sb")
        nc.sync.dma_start(out=t, in_=io[nm])
        w_sb[nm] = t
    woT_sb = const.tile([128, D], F16, name="woT_sb")
    nc.sync.dma_start(out=woT_sb, in_=io["woT"])
    cmask_sb = const.tile([128, 4, 2 * 512], F16, name="cmask_sb")
    nc.sync.dma_start(out=cmask_sb, in_=io["cmask"])
    pbias_sb = const.tile([128, B, NJT], F32, name="pbias_sb")
    nc.sync.dma_start(out=pbias_sb, in_=io["pad_bias"])

    qhT_sb = const.tile([128, BS], F16, name="qhT_sb")  # [c2, b*S + i]
    khT_sb = const.tile([128, BS], F16, name="khT_sb")
    vaug_sb = const.tile([128, B, H_LOC, NJT, DK + 1], F16, name="vaug_sb")
    nc.vector.memset(vaug_sb[:, :, :, :, DK : DK + 1], 1.0)
    aT_sb = const.tile([128, BS], F16, name="aT_sb")
    araw = {
        b: const.tile([DK + 1, H_LOC, S], F16, name=f"araw_{b}") for b in range(B)
    }

    # ---- phase A as a list of work units (closures) for feeding into B ----
    def a_units(ss, rep):
        xts = {}
        units = []

        def load_unit():
            for nm in ("q", "k", "v"):
                xT = io["x" + nm + "T"]
                for kk in range(8):
                    xt = xpool.tile(
                        [128, 1024], F16, name=f"x{nm}_{rep}_{ss}_{kk}", tag="xt"
                    )
                    nc.sync.dma_start(
                        out=xt,
                        in_=xT[
                            kk * 128 : (kk + 1) * 128, ss * 1024 : (ss + 1) * 1024
                        ],
                    )
                    xts[nm, kk] = xt

        units.append(load_unit)

        def qk_unit(nm, sc):
            def run():
                ps = psum_mm.tile(
                    [128, 512], F32, name=f"psA{nm}_{rep}_{ss}_{sc}", tag="mm"
                )
                for kk in range(8):
                    nc.tensor.matmul(
                        ps,
                        lhsT=w_sb["w" + nm][:, kk, :],
                        rhs=xts[nm, kk][:, sc * 512 : (sc + 1) * 512],
                        start=(kk == 0),
                        stop=(kk == 7),
                    )
                col = ss * 1024 + sc * 512
                outbuf = {"q": qhT_sb, "k": khT_sb}[nm]
                nc.vector.tensor_copy(outbuf[:, col : col + 512], ps)

            return run

        for nm in ("q", "k"):
            for sc in range(2):
                units.append(qk_unit(nm, sc))

        vps = {}

        def v_unit(sp, i4):
            def run():
                if i4 == 0:
                    vps[sp] = psum_mm.tile(
                        [128, 4, 128], F32, name=f"psV_{rep}_{ss}_{sp}", tag="mm"
                    )
                ps = vps[sp]
                sl = sp * 4 + i4
                for kk in range(8):
                    nc.tensor.matmul(
                        ps[:, i4, :],
                        lhsT=xts["v", kk][:, sl * 128 : (sl + 1) * 128],
                        rhs=w_sb["wv"][:, kk, :],
                        start=(kk == 0),
                        stop=(kk == 7),
                    )
                if i4 == 3:
                    sch0 = ss * 8 + sp * 4
                    b, jt0 = divmod(sch0, NJT)
                    nc.vector.tensor_copy(
                        vaug_sb[:, b, :, jt0 : jt0 + 4, 0:DK].rearrange(
                            "p h j c -> p j h c"
                        ),
                        ps.rearrange("p j (h c) -> p j h c", h=H_LOC),
                    )

            return run

        for sp in range(2):
            for i4 in range(4):
                units.append(v_unit(sp, i4))
        return units

    # ---- divide: split so the DRAM round-trip latency of the denominator
    # broadcast never blocks the in-order DVE/Sync queues: `pre` (Pool DMAs)
    # right after the araw copy, `post` (DVE) a few jt-iterations later.
    def divide_pre(b, ic, rep):
        dchunk = slice(ic * 512, (ic + 1) * 512)
        rrow = io["rscratch"][b * 4 + ic : b * 4 + ic + 1, :]
        nc.gpsimd.dma_start(out=rrow, in_=araw[b][DK : DK + 1, :, dchunk])
        # fp16 DRAM row -> fp16 SBUF with partition replication (casting in
        # a swdge DMA is software-slow; upcast on DVE in divide_post instead)
        den = mpool.tile([DK, H_LOC, 512], F16, name=f"den_{rep}{b}{ic}", tag="den")
        dbcast = bass.AP(
            tensor=rrow.tensor,
            offset=rrow.offset,
            ap=[[0, DK], [512, H_LOC], [1, 512]],
        )
        nc.gpsimd.dma_start(out=den, in_=dbcast)
        return den

    def divide_post(b, ic, den, rep):
        dchunk = slice(ic * 512, (ic + 1) * 512)
        dcols = slice(b * S + ic * 512, b * S + (ic + 1) * 512)
        den32 = mpool.tile([DK, H_LOC, 512], F32, name=f"d32_{rep}{b}{ic}", tag="d32")
        nc.vector.tensor_copy(den32, den)
        denr = mpool.tile([DK, H_LOC, 512], F32, name=f"dr_{rep}{b}{ic}", tag="denr")
        nc.vector.reciprocal_approx_fast(denr, den32)
        nc.vector.tensor_mul(
            aT_sb[0:DK, dcols], araw[b][0:DK, 0, dchunk], denr[:, 0, :]
        )
        tmpa = mpool.tile([DK, 512], F16, name=f"ta_{rep}{b}{ic}", tag="ta")
        nc.vector.tensor_mul(tmpa, araw[b][0:DK, 1, dchunk], denr[:, 1, :])
        # partition remap 0:64 -> 64:128 via SBUF->SBUF DMA (Pool queue)
        nc.gpsimd.dma_start(out=aT_sb[DK : 2 * DK, dcols], in_=tmpa)

    def phase_c(b, schs, rep, tail=False):
        for sch in schs:
            ysb = ypool.tile([128, 1024], F16, name=f"ysb_{rep}{b}{sch}", tag="ysb")
            for eh in range(2):
                py = psum_mm.tile(
                    [128, 512], F32, name=f"psC_{rep}{b}{sch}{eh}", tag="mm"
                )
                nc.tensor.matmul(
                    py,
                    lhsT=aT_sb[:, b * S + sch * 128 : b * S + (sch + 1) * 128],
                    rhs=woT_sb[:, eh * 512 : (eh + 1) * 512],
                    start=True,
                    stop=True,
                )
                if tail and eh == 0:
                    nc.scalar.copy(ysb[:, 0:512], py)
                else:
                    nc.vector.tensor_copy(ysb[:, eh * 512 : (eh + 1) * 512], py)
            r0 = b * S + sch * 128
            nc.gpsimd.dma_start(out=io["y"][r0 : r0 + 128, :], in_=ysb)

    # ---- phase B: attention (head pair together) + chunked division + C ----
    # `post_in` is the previous phase's final (b, ic, den) divide_post, run at
    # slot 2 here so its DRAM round-trip never blocks the DVE queue. Own
    # divide_posts run ~5 slots after their pre. `c_feed` is a list of
    # (min_slot, b, sch) output tiles, emitted once their aT chunk is ready.
    # `feed` holds phase-A units, one per slot, front-loaded.
    POST_K = {0: 8, 1: 16, 2: 28}  # slot for divide_post of ic 0..2

    def phase_b(b, rep, post_in=None, c_feed=(), feed=()):
        feed = list(feed)
        c_items = sorted(c_feed)
        pending = {}  # ic -> den
        k = 0
        for ic in range(4):  # query chunks of 512
            njt = 4 * (ic + 1)  # causal: keys up to end of this query chunk
            ichunk = slice(ic * 512, (ic + 1) * 512)
            po = psum_o.tile(
                [DK + 1, H_LOC, 512], F32, name=f"po_{rep}{b}{ic}", tag="po"
            )
            for jt in range(njt):
                if feed:
                    feed.pop(0)()  # one phase-A unit per slot, front-loaded
                if post_in is not None and k == 2:
                    b_p, ic_p, den = post_in
                    divide_post(b_p, ic_p, den, rep)
                    post_in = None
                for ic_p, kk in list(POST_K.items()):
                    if k == kk and ic_p in pending:
                        divide_post(b, ic_p, pending.pop(ic_p), rep)
                while c_items and c_items[0][0] <= k:
                    _, b_c, sch = c_items.pop(0)
                    phase_c(b_c, [sch], rep)
                    break  # at most one C tile per slot
                k += 1
                jcols = slice(b * S + jt * 128, b * S + (jt + 1) * 128)
                # diagonal tiles: columns i < (jt-4ic)*128 are fully masked --
                # skip them in the scores matmul, exp, mask, and PV.
                o = jt - 4 * ic
                lo = o * 128 if o >= 0 else 0
                ps = psum_s.tile(
                    [128, H_LOC, 512], F32, name=f"psS_{rep}{b}{ic}{jt}", tag="ps"
                )
                for h in range(H_LOC):
                    r0 = DK * h
                    nc.tensor.matmul(
                        ps[:, h, lo:512],
                        lhsT=khT_sb[r0 : r0 + DK, jcols],
                        rhs=qhT_sb[
                            r0 : r0 + DK,
                            b * S + ic * 512 + lo : b * S + (ic + 1) * 512,
                        ],
                        start=True,
                        stop=True,
                    )
                pe = ppool.tile(
                    [128, H_LOC, 512], F16, name=f"pe_{rep}{b}{ic}{jt}", tag="pe"
                )
                nc.scalar.activation(
                    pe[:, :, lo:512],
                    ps[:, :, lo:512],
                    FT.Exp,
                    bias=pbias_sb[:, b, jt : jt + 1],
                    scale=0.125,
                )
                if o >= 0:  # diagonal tile: causal zero-mask on the live slice
                    cmv = cmask_sb[:, o, :].rearrange("p (h i) -> p h i", h=H_LOC)
                    nc.vector.tensor_mul(
                        pe[:, :, lo:512], pe[:, :, lo:512], cmv[:, :, lo:512]
                    )
                for h in range(H_LOC):
                    nc.tensor.matmul(
                        po[:, h, lo:512],
                        lhsT=vaug_sb[:, b, h, jt, :],
                        rhs=pe[:, h, lo:512],
                        start=(jt == 0),
                        stop=(jt == njt - 1),
                    )
            # one quick copy releases po
            nc.scalar.copy(araw[b][:, :, ichunk], po)
            pending[ic] = divide_pre(b, ic, rep)
        for u in feed:  # leftovers (feed longer than jt count)
            u()
        for _, b_c, sch in c_items:  # leftover C tiles
            phase_c(b_c, [sch], rep)
        return (b, 3, pending.pop(3))

    # ---- main schedule: B phases host A (and C) work units ----
    # Steady state per rep: B0 hosts A(ss2)+A(ss3) + C(b1 prev-rep, 12:16) +
    # C(b0, 0:12); B1 hosts next rep's A(ss0)+A(ss1) + C(b0, 12:16) +
    # C(b1, 0:12). Each phase's final divide_post is carried into the next
    # phase so its DRAM round-trip latency is hidden.
    def c_sched(b_new, b_old):
        # own-b tiles become ready ~5 slots after their divide_post (the
        # post->recip->muls->remap chain is ~4us)
        items = [(POST_K[sch // 4] + 5, b_new, sch) for sch in range(12)]
        # other-b ic3 tiles: ready a few slots after the carried post (slot 2)
        items += [(7 + (sch - 12), b_old, sch) for sch in range(12, 16)]
        return items

    post = None
    for r in range(reps):
        if r == 0:
            for u in a_units(0, 0) + a_units(1, 0):
                u()
        cf0 = c_sched(0, 1) if r > 0 else [
            (POST_K[sch // 4] + 5, 0, sch) for sch in range(12)
        ]
        post = phase_b(0, r, post_in=post, c_feed=cf0,
                       feed=a_units(2, r) + a_units(3, r))
        nxt = a_units(0, r + 1) + a_units(1, r + 1) if r + 1 < reps else []
        post = phase_b(1, r, post_in=post, c_feed=c_sched(1, 0), feed=nxt)
    # tail: final divide_post + last output tiles
    b_p, ic_p, den = post
    divide_post(b_p, ic_p, den, reps)
    phase_c(1, range(12, 16), reps, tail=True)


def get_nc():
    if "nc" not in _CACHE:
        _CACHE["nc"] = _build_nc()
    return _CACHE["nc"]


def prep_inputs(q, k, v, mask, Wq, Wk, Wv, Wo):
    """Host-side shard prep: transposes, fp16 casts, per-core weight slices."""
    q = np.asarray(q, dtype=np.float32).reshape(BS, D)
    k = np.asarray(k, dtype=np.float32).reshape(BS, D)
    v = np.asarray(v, dtype=np.float32).reshape(BS, D)
    mask = np.asarray(mask)
    Wq, Wk, Wv, Wo = (np.asarray(w, dtype=np.float32) for w in (Wq, Wk, Wv, Wo))

    xqT = np.ascontiguousarray(q.T).astype(np.float16)
    xkT = np.ascontiguousarray(k.T).astype(np.float16)
    xvT = np.ascontiguousarray(v.T).astype(np.float16)

    pb = np.where(mask == 0, np.float32(-1e9), np.float32(0.0)).astype(np.float32)
    # [B, S] -> [128, B, S//128]  (partition = j % 128, col = key tile)
    pad_bias = np.ascontiguousarray(pb.reshape(B, S // 128, 128).transpose(2, 0, 1))

    p_idx = np.arange(128)[:, None, None]
    o_idx = np.arange(4)[None, :, None]
    q_idx = np.arange(512)[None, None, :]
    cm = (q_idx >= o_idx * 128 + p_idx).astype(np.float16)  # [128, 4, 512]
    cmask = np.ascontiguousarray(
        np.broadcast_to(cm[:, :, None, :], (128, 4, H_LOC, 512)).reshape(
            128, 4, H_LOC * 512
        )
    )

    def wslice(Wmat, c):
        ws = Wmat[c * M_LOC : (c + 1) * M_LOC, :]  # [128 out, 1024 in]
        # -> [p(=d%128), kk(=d//128), c2]
        return np.ascontiguousarray(
            ws.T.reshape(8, 128, M_LOC).transpose(1, 0, 2)
        ).astype(np.float16)

    in_maps = []
    for c in range(N_CORES):
        woT_c = np.ascontiguousarray(Wo[:, c * M_LOC : (c + 1) * M_LOC].T).astype(
            np.float16
        )
        in_maps.append(
            {
                "xqT": xqT,
                "xkT": xkT,
                "xvT": xvT,
                "wq": wslice(Wq, c),
                "wk": wslice(Wk, c),
                "wv": wslice(Wv, c),
                "woT": woT_c,
                "pad_bias": pad_bias,
                "cmask": cmask,
            }
        )
    return in_maps


def gather_output(results):
    acc = np.zeros((BS, D), dtype=np.float32)
    for r in results:
        acc += r["y_partial"].astype(np.float32)
    return acc.reshape(B, S, D)


def kernel(q, k, v, mask, Wq, Wk, Wv, Wo):
    nc = get_nc()
    in_maps = prep_inputs(q, k, v, mask, Wq, Wk, Wv, Wo)
    res = run_bass_kernel_spmd(nc, in_maps, core_ids=list(range(N_CORES)))
    return gather_output(res.results)


# revision 41
# speedup vs baseline: 2.1983x; 2.1983x over previous
"""Multi-head causal attention (B=2, S=2048, D=1024, H=16) on 8 trn2 NeuronCores.

Sharding: 2 heads per core (tensor-parallel over the 16 heads). Each core
receives the full (host-pre-transposed, fp16) activations plus its own slice
of the projection weights, computes

    qhT/khT = (Wq_l @ x.T)      [128, B*S]   (head dim on partitions)
    vh      = x @ Wv_l.T        per 128-row chunk, stored [j, c] + ones column
    S_T     = kh @ qh.T / 8     per (b, h-pair), keys on partitions
    P       = exp(S_T + pad_bias) * causal_mask
    attT    = (v_aug.T @ P) -> rows 0:64 = att.T, row 64 = softmax denominator
    aT      = attT / denom      (denominator broadcast via DMA replication)
    y_part  = A_local @ Wo[:, mslice].T     [B*S, D] partial sums

The host sums the 8 partial outputs (standard row-parallel unshard) and
reshapes to [B, S, D] float32.

Schedule: the exp-bound attention loops (phase B) are fed with phase-A
projection work units so the tensor engine fills its exp-wait bubbles:
B(b=0) hosts A(ss2)+A(ss3); B(b=1) hosts the NEXT repetition's A(ss0)+A(ss1)
plus the output-projection (phase C) tiles. DMA queues: Sync streams x tiles
only; the divide round-trip and y writes ride the idle Pool (gpsimd) queue.
"""

import os
import sys

import numpy as np

try:
    import concourse.bass as bass
except ImportError:  # fallback if sitecustomize did not add the repo
    for _p in ("/opt/trn_rl_repo", "/root/.axon_site/_ro/trn_rl_repo"):
        if os.path.isdir(_p) and _p not in sys.path:
            sys.path.insert(0, _p)
    import concourse.bass as bass  # noqa: F401

import concourse.tile as tile
from concourse import bacc, mybir
from concourse.bass_utils import run_bass_kernel_spmd

B, S, D, H, DK = 2, 2048, 1024, 16, 64
BS = B * S                # 4096
N_CORES = 8
H_LOC = H // N_CORES      # 2 heads per core
M_LOC = H_LOC * DK        # 128 local concat dim
NJT = S // 128            # 16 key tiles per batch

F16 = mybir.dt.float16
F32 = mybir.dt.float32
FT = mybir.ActivationFunctionType

_CACHE = {}


def _build_nc(reps=1):
    """Build + compile the per-core Bass program (identical across cores).

    reps > 1 repeats the whole (idempotent) body back-to-back inside one
    NEFF — used by test.py to time one rep via the slope between NEFFs.
    """
    from contextlib import ExitStack

    nc = bacc.Bacc(
        "TRN2", target_bir_lowering=False, debug=False, enable_asserts=False
    )

    io = {}
    for nm in ("xqT", "xkT", "xvT"):
        io[nm] = nc.dram_tensor(nm, [D, BS], F16, kind="ExternalInput").ap()
    for nm in ("wq", "wk", "wv"):
        io[nm] = nc.dram_tensor(nm, [128, 8, 128], F16, kind="ExternalInput").ap()
    io["woT"] = nc.dram_tensor("woT", [128, D], F16, kind="ExternalInput").ap()
    io["pad_bias"] = nc.dram_tensor(
        "pad_bias", [128, B, NJT], F32, kind="ExternalInput"
    ).ap()
    # causal masks for the 4 diagonal offsets, duplicated along the head pair:
    # [128, 4, 2*512]
    io["cmask"] = nc.dram_tensor(
        "cmask", [128, 4, 2 * 512], F16, kind="ExternalInput"
    ).ap()
    io["y"] = nc.dram_tensor("y_partial", [BS, D], F16, kind="ExternalOutput").ap()
    # scratch for the softmax-denominator broadcast (DRAM allows 0-step APs)
    io["rscratch"] = nc.dram_tensor("rscratch", [B * 4, H_LOC * 512], F16).ap()

    with tile.TileContext(nc) as tc, ExitStack() as ctx:
        pools = {
            "const": ctx.enter_context(tc.tile_pool(name="const", bufs=1)),
            "xpool": ctx.enter_context(tc.tile_pool(name="xpool", bufs=36)),
            "ppool": ctx.enter_context(tc.tile_pool(name="ppool", bufs=6)),
            "mpool": ctx.enter_context(tc.tile_pool(name="mpool", bufs=2)),
            "ypool": ctx.enter_context(tc.tile_pool(name="ypool", bufs=3)),
            # PSUM budget (8 banks): ps 2x2 + po 1x2 + mm 2x1 = 8
            "psum_s": ctx.enter_context(
                tc.tile_pool(name="psum_s", bufs=2, space="PSUM")
            ),
            "psum_o": ctx.enter_context(
                tc.tile_pool(name="psum_o", bufs=1, space="PSUM")
            ),
            "psum_mm": ctx.enter_context(
                tc.tile_pool(name="psum_mm", bufs=2, space="PSUM")
            ),
        }
        _program(pools, tc, io, reps)
    nc.compile()
    return nc


def _program(pools, tc, io, reps):
    nc = tc.nc

    const = pools["const"]
    xpool = pools["xpool"]
    ppool = pools["ppool"]
    mpool = pools["mpool"]
    ypool = pools["ypool"]
    psum_s = pools["psum_s"]
    psum_o = pools["psum_o"]
    psum_mm = pools["psum_mm"]

    # ---- constants / persistent buffers (loaded once for all reps) ----
    w_sb = {}
    for nm in ("wq", "wk", "wv"):
        t = const.tile([128, 8, 128], F16, name=f"{nm}_sb")
        nc.sync.dma_start(out=t, in_=io[nm])
        w_sb[nm] = t
    woT_sb = const.tile([128, D], F16, name="woT_sb")
    nc.sync.dma_start(out=woT_sb, in_=io["woT"])
    cmask_sb = const.tile([128, 4, 2 * 512], F16, name="cmask_sb")
    nc.sync.dma_start(out=cmask_sb, in_=io["cmask"])
    pbias_sb = const.tile([128, B, NJT], F32, name="pbias_sb")
    nc.sync.dma_start(out=pbias_sb, in_=io["pad_bias"])

    qhT_sb = const.tile([128, BS], F16, name="qhT_sb")  # [c2, b*S + i]
    khT_sb = const.tile([128, BS], F16, name="khT_sb")
    vaug_sb = const.tile([128, B, H_LOC, NJT, DK + 1], F16, name="vaug_sb")
    nc.vector.memset(vaug_sb[:, :, :, :, DK : DK + 1], 1.0)
    aT_sb = const.tile([128, BS], F16, name="aT_sb")
    araw = {
        b: const.tile([DK + 1, H_LOC, S], F16, name=f"araw_{b}") for b in range(B)
    }

    # ---- phase A as a list of work units (closures) for feeding into B ----
    def a_units(ss, rep):
        xts = {}
        units = []

        def load_unit():
            for nm in ("q", "k", "v"):
                xT = io["x" + nm + "T"]
                for kk in range(8):
                    xt = xpool.tile(
                        [128, 1024], F16, name=f"x{nm}_{rep}_{ss}_{kk}", tag="xt"
                    )
                    nc.sync.dma_start(
                        out=xt,
                        in_=xT[
                            kk * 128 : (kk + 1) * 128, ss * 1024 : (ss + 1) * 1024
                        ],
                    )
                    xts[nm, kk] = xt

        units.append(load_unit)

        def qk_unit(nm, sc):
            def run():
                ps = psum_mm.tile(
                    [128, 512], F32, name=f"psA{nm}_{rep}_{ss}_{sc}", tag="mm"
                )
                for kk in range(8):
                    nc.tensor.matmul(
                        ps,
                        lhsT=w_sb["w" + nm][:, kk, :],
                        rhs=xts[nm, kk][:, sc * 512 : (sc + 1) * 512],
                        start=(kk == 0),
                        stop=(kk == 7),
                    )
                col = ss * 1024 + sc * 512
                outbuf = {"q": qhT_sb, "k": khT_sb}[nm]
                nc.vector.tensor_copy(outbuf[:, col : col + 512], ps)

            return run

        for nm in ("q", "k"):
            for sc in range(2):
                units.append(qk_unit(nm, sc))

        vps = {}

        def v_unit(sp, half):
            def run():
                if half == 0:
                    vps[sp] = psum_mm.tile(
                        [128, 4, 128], F32, name=f"psV_{rep}_{ss}_{sp}", tag="mm"
                    )
                ps = vps[sp]
                for i4 in (2 * half, 2 * half + 1):
                    sl = sp * 4 + i4
                    for kk in range(8):
                        nc.tensor.matmul(
                            ps[:, i4, :],
                            lhsT=xts["v", kk][:, sl * 128 : (sl + 1) * 128],
                            rhs=w_sb["wv"][:, kk, :],
                            start=(kk == 0),
                            stop=(kk == 7),
                        )
                if half == 1:
                    sch0 = ss * 8 + sp * 4
                    b, jt0 = divmod(sch0, NJT)
                    nc.vector.tensor_copy(
                        vaug_sb[:, b, :, jt0 : jt0 + 4, 0:DK].rearrange(
                            "p h j c -> p j h c"
                        ),
                        ps.rearrange("p j (h c) -> p j h c", h=H_LOC),
                    )

            return run

        for sp in range(2):
            for half in range(2):
                units.append(v_unit(sp, half))
        return units

    # ---- divide: split so the DRAM round-trip latency of the denominator
    # broadcast never blocks the in-order DVE/Sync queues: `pre` (Pool DMAs)
    # right after the araw copy, `post` (DVE) a few jt-iterations later.
    def divide_pre(b, ic, rep):
        dchunk = slice(ic * 512, (ic + 1) * 512)
        rrow = io["rscratch"][b * 4 + ic : b * 4 + ic + 1, :]
        nc.gpsimd.dma_start(out=rrow, in_=araw[b][DK : DK + 1, :, dchunk])
        # fp16 DRAM row -> fp16 SBUF with partition replication (casting in
        # a swdge DMA is software-slow; upcast on DVE in divide_post instead)
        den = mpool.tile([DK, H_LOC, 512], F16, name=f"den_{rep}{b}{ic}", tag="den")
        dbcast = bass.AP(
            tensor=rrow.tensor,
            offset=rrow.offset,
            ap=[[0, DK], [512, H_LOC], [1, 512]],
        )
        nc.gpsimd.dma_start(out=den, in_=dbcast)
        return den

    def divide_post(b, ic, den, rep):
        dchunk = slice(ic * 512, (ic + 1) * 512)
        dcols = slice(b * S + ic * 512, b * S + (ic + 1) * 512)
        den32 = mpool.tile([DK, H_LOC, 512], F32, name=f"d32_{rep}{b}{ic}", tag="d32")
        nc.vector.tensor_copy(den32, den)
        denr = mpool.tile([DK, H_LOC, 512], F32, name=f"dr_{rep}{b}{ic}", tag="denr")
        nc.vector.reciprocal_approx_fast(denr, den32)
        nc.vector.tensor_mul(
            aT_sb[0:DK, dcols], araw[b][0:DK, 0, dchunk], denr[:, 0, :]
        )
        tmpa = mpool.tile([DK, 512], F16, name=f"ta_{rep}{b}{ic}", tag="ta")
        nc.vector.tensor_mul(tmpa, araw[b][0:DK, 1, dchunk], denr[:, 1, :])
        # partition remap 0:64 -> 64:128 via SBUF->SBUF DMA (Pool queue)
        nc.gpsimd.dma_start(out=aT_sb[DK : 2 * DK, dcols], in_=tmpa)

    def phase_c(b, schs, rep, tail=False):
        for sch in schs:
            ysb = ypool.tile([128, 1024], F16, name=f"ysb_{rep}{b}{sch}", tag="ysb")
            for eh in range(2):
                py = psum_mm.tile(
                    [128, 512], F32, name=f"psC_{rep}{b}{sch}{eh}", tag="mm"
                )
                nc.tensor.matmul(
                    py,
                    lhsT=aT_sb[:, b * S + sch * 128 : b * S + (sch + 1) * 128],
                    rhs=woT_sb[:, eh * 512 : (eh + 1) * 512],
                    start=True,
                    stop=True,
                )
                if tail and eh == 0:
                    nc.scalar.copy(ysb[:, 0:512], py)
                else:
                    nc.vector.tensor_copy(ysb[:, eh * 512 : (eh + 1) * 512], py)
            r0 = b * S + sch * 128
            nc.gpsimd.dma_start(out=io["y"][r0 : r0 + 128, :], in_=ysb)

    # ---- phase B: attention (head pair together) + chunked division + C ----
    # `post_in` is the previous phase's final (b, ic, den) divide_post, run at
    # slot 2 here so its DRAM round-trip never blocks the DVE queue. Own
    # divide_posts run ~5 slots after their pre. `c_feed` is a list of
    # (min_slot, b, sch) output tiles, emitted once their aT chunk is ready.
    # `feed` holds phase-A units, one per slot, front-loaded.
    POST_K = {0: 8, 1: 16, 2: 28}  # slot for divide_post of ic 0..2

    def phase_b(b, rep, post_in=None, c_feed=(), feed=()):
        feed = list(feed)
        c_items = sorted(c_feed)
        pending = {}  # ic -> den
        k = 0
        for ic in range(4):  # query chunks of 512
            njt = 4 * (ic + 1)  # causal: keys up to end of this query chunk
            ichunk = slice(ic * 512, (ic + 1) * 512)
            po = psum_o.tile(
                [DK + 1, H_LOC, 512], F32, name=f"po_{rep}{b}{ic}", tag="po"
            )
            for jt in range(njt):
                if post_in is not None and k == 2:
                    b_p, ic_p, den = post_in
                    divide_post(b_p, ic_p, den, rep)
                    post_in = None
                for ic_p, kk in list(POST_K.items()):
                    if k == kk and ic_p in pending:
                        divide_post(b, ic_p, pending.pop(ic_p), rep)
                # one PE-filler event per slot: a due C tile, else an A unit
                if c_items and c_items[0][0] <= k:
                    _, b_c, sch = c_items.pop(0)
                    phase_c(b_c, [sch], rep)
                elif feed:
                    feed.pop(0)()
                k += 1
                jcols = slice(b * S + jt * 128, b * S + (jt + 1) * 128)
                # diagonal tiles: columns i < (jt-4ic)*128 are fully masked --
                # skip them in the scores matmul, exp, mask, and PV.
                o = jt - 4 * ic
                lo = o * 128 if o >= 0 else 0
                ps = psum_s.tile(
                    [128, H_LOC, 512], F32, name=f"psS_{rep}{b}{ic}{jt}", tag="ps"
                )
                for h in range(H_LOC):
                    r0 = DK * h
                    nc.tensor.matmul(
                        ps[:, h, lo:512],
                        lhsT=khT_sb[r0 : r0 + DK, jcols],
                        rhs=qhT_sb[
                            r0 : r0 + DK,
                            b * S + ic * 512 + lo : b * S + (ic + 1) * 512,
                        ],
                        start=True,
                        stop=True,
                    )
                pe = ppool.tile(
                    [128, H_LOC, 512], F16, name=f"pe_{rep}{b}{ic}{jt}", tag="pe"
                )
                nc.scalar.activation(
                    pe[:, :, lo:512],
                    ps[:, :, lo:512],
                    FT.Exp,
                    bias=pbias_sb[:, b, jt : jt + 1],
                    scale=0.125,
                )
                if o >= 0:  # diagonal tile: causal zero-mask on the live slice
                    cmv = cmask_sb[:, o, :].rearrange("p (h i) -> p h i", h=H_LOC)
                    nc.vector.tensor_mul(
                        pe[:, :, lo:512], pe[:, :, lo:512], cmv[:, :, lo:512]
                    )
                for h in range(H_LOC):
                    nc.tensor.matmul(
                        po[:, h, lo:512],
                        lhsT=vaug_sb[:, b, h, jt, :],
                        rhs=pe[:, h, lo:512],
                        start=(jt == 0),
                        stop=(jt == njt - 1),
                    )
            # one quick copy releases po
            nc.scalar.copy(araw[b][:, :, ichunk], po)
            pending[ic] = divide_pre(b, ic, rep)
        for u in feed:  # leftovers (feed longer than jt count)
            u()
        for _, b_c, sch in c_items:  # leftover C tiles
            phase_c(b_c, [sch], rep)
        return (b, 3, pending.pop(3))

    # ---- main schedule: B phases host A (and C) work units ----
    # Steady state per rep: B0 hosts A(ss2)+A(ss3) + C(b1 prev-rep, 12:16) +
    # C(b0, 0:12); B1 hosts next rep's A(ss0)+A(ss1) + C(b0, 12:16) +
    # C(b1, 0:12). Each phase's final divide_post is carried into the next
    # phase so its DRAM round-trip latency is hidden.
    def c_sched(b_new, b_old):
        # own-b tiles become ready ~5 slots after their divide_post (the
        # post->recip->muls->remap chain is ~4us); stagger to avoid DVE bursts
        items = [
            (POST_K[sch // 4] + 5 + 2 * (sch % 4), b_new, sch) for sch in range(12)
        ]
        # other-b ic3 tiles: ready a few slots after the carried post (slot 2)
        items += [(7 + 2 * (sch - 12), b_old, sch) for sch in range(12, 16)]
        return items

    post = None
    for r in range(reps):
        if r == 0:
            for u in a_units(0, 0) + a_units(1, 0):
                u()
        cf0 = c_sched(0, 1) if r > 0 else [
            (POST_K[sch // 4] + 5 + 2 * (sch % 4), 0, sch) for sch in range(12)
        ]
        post = phase_b(0, r, post_in=post, c_feed=cf0,
                       feed=a_units(2, r) + a_units(3, r))
        nxt = a_units(0, r + 1) + a_units(1, r + 1) if r + 1 < reps else []
        post = phase_b(1, r, post_in=post, c_feed=c_sched(1, 0), feed=nxt)
    # tail: final divide_post + last output tiles
    b_p, ic_p, den = post
    divide_post(b_p, ic_p, den, reps)
    phase_c(1, range(12, 16), reps, tail=True)


def get_nc():
    if "nc" not in _CACHE:
        _CACHE["nc"] = _build_nc()
    return _CACHE["nc"]


def prep_inputs(q, k, v, mask, Wq, Wk, Wv, Wo):
    """Host-side shard prep: transposes, fp16 casts, per-core weight slices."""
    q = np.asarray(q, dtype=np.float32).reshape(BS, D)
    k = np.asarray(k, dtype=np.float32).reshape(BS, D)
    v = np.asarray(v, dtype=np.float32).reshape(BS, D)
    mask = np.asarray(mask)
    Wq, Wk, Wv, Wo = (np.asarray(w, dtype=np.float32) for w in (Wq, Wk, Wv, Wo))

    xqT = np.ascontiguousarray(q.T).astype(np.float16)
    xkT = np.ascontiguousarray(k.T).astype(np.float16)
    xvT = np.ascontiguousarray(v.T).astype(np.float16)

    pb = np.where(mask == 0, np.float32(-1e9), np.float32(0.0)).astype(np.float32)
    # [B, S] -> [128, B, S//128]  (partition = j % 128, col = key tile)
    pad_bias = np.ascontiguousarray(pb.reshape(B, S // 128, 128).transpose(2, 0, 1))

    p_idx = np.arange(128)[:, None, None]
    o_idx = np.arange(4)[None, :, None]
    q_idx = np.arange(512)[None, None, :]
    cm = (q_idx >= o_idx * 128 + p_idx).astype(np.float16)  # [128, 4, 512]
    cmask = np.ascontiguousarray(
        np.broadcast_to(cm[:, :, None, :], (128, 4, H_LOC, 512)).reshape(
            128, 4, H_LOC * 512
        )
    )

    def wslice(Wmat, c):
        ws = Wmat[c * M_LOC : (c + 1) * M_LOC, :]  # [128 out, 1024 in]
        # -> [p(=d%128), kk(=d//128), c2]
        return np.ascontiguousarray(
            ws.T.reshape(8, 128, M_LOC).transpose(1, 0, 2)
        ).astype(np.float16)

    in_maps = []
    for c in range(N_CORES):
        woT_c = np.ascontiguousarray(Wo[:, c * M_LOC : (c + 1) * M_LOC].T).astype(
            np.float16
        )
        in_maps.append(
            {
                "xqT": xqT,
                "xkT": xkT,
                "xvT": xvT,
                "wq": wslice(Wq, c),
                "wk": wslice(Wk, c),
                "wv": wslice(Wv, c),
                "woT": woT_c,
                "pad_bias": pad_bias,
                "cmask": cmask,
            }
        )
    return in_maps


def gather_output(results):
    acc = np.zeros((BS, D), dtype=np.float32)
    for r in results:
        acc += r["y_partial"].astype(np.float32)
    return acc.reshape(B, S, D)


def kernel(q, k, v, mask, Wq, Wk, Wv, Wo):
    nc = get_nc()
    in_maps = prep_inputs(q, k, v, mask, Wq, Wk, Wv, Wo)
    res = run_bass_kernel_spmd(nc, in_maps, core_ids=list(range(N_CORES)))
    return gather_output(res.results)


# revision 42
# speedup vs baseline: 2.5329x; 1.1522x over previous
"""Multi-head causal attention (B=2, S=2048, D=1024, H=16) on 8 trn2 NeuronCores.

Sharding: 2 heads per core (tensor-parallel over the 16 heads). Each core
receives the full (host-pre-transposed, fp16) activations plus its own slice
of the projection weights, computes

    qhT/khT = (Wq_l @ x.T)      [128, B*S]   (head dim on partitions)
    vh      = x @ Wv_l.T        per 128-row chunk, stored [j, c] + ones column
    S_T     = kh @ qh.T / 8     per (b, h-pair), keys on partitions
    P       = exp(S_T + pad_bias) * causal_mask
    attT    = (v_aug.T @ P) -> rows 0:64 = att.T, row 64 = softmax denominator
    aT      = attT / denom      (denominator broadcast via DMA replication)
    y_part  = A_local @ Wo[:, mslice].T     [B*S, D] partial sums

The host sums the 8 partial outputs (standard row-parallel unshard) and
reshapes to [B, S, D] float32.

Schedule: the exp-bound attention loops (phase B) are fed with phase-A
projection work units so the tensor engine fills its exp-wait bubbles:
B(b=0) hosts A(ss2)+A(ss3); B(b=1) hosts the NEXT repetition's A(ss0)+A(ss1)
plus the output-projection (phase C) tiles. DMA queues: Sync streams x tiles
only; the divide round-trip and y writes ride the idle Pool (gpsimd) queue.
"""

import os
import sys

import numpy as np

try:
    import concourse.bass as bass
except ImportError:  # fallback if sitecustomize did not add the repo
    for _p in ("/opt/trn_rl_repo", "/root/.axon_site/_ro/trn_rl_repo"):
        if os.path.isdir(_p) and _p not in sys.path:
            sys.path.insert(0, _p)
    import concourse.bass as bass  # noqa: F401

import concourse.tile as tile
from concourse import bacc, mybir
from concourse.bass_utils import run_bass_kernel_spmd

B, S, D, H, DK = 2, 2048, 1024, 16, 64
BS = B * S                # 4096
N_CORES = 8
H_LOC = H // N_CORES      # 2 heads per core
M_LOC = H_LOC * DK        # 128 local concat dim
NJT = S // 128            # 16 key tiles per batch

F16 = mybir.dt.float16
F32 = mybir.dt.float32
FT = mybir.ActivationFunctionType

_CACHE = {}


def _build_nc(reps=1):
    """Build + compile the per-core Bass program (identical across cores).

    reps > 1 repeats the whole (idempotent) body back-to-back inside one
    NEFF — used by test.py to time one rep via the slope between NEFFs.
    """
    from contextlib import ExitStack

    nc = bacc.Bacc(
        "TRN2", target_bir_lowering=False, debug=False, enable_asserts=False
    )

    io = {}
    for nm in ("xqT", "xkT", "xvT"):
        io[nm] = nc.dram_tensor(nm, [D, BS], F16, kind="ExternalInput").ap()
    for nm in ("wq", "wk", "wv"):
        io[nm] = nc.dram_tensor(nm, [128, 8, 128], F16, kind="ExternalInput").ap()
    io["woT"] = nc.dram_tensor("woT", [128, D], F16, kind="ExternalInput").ap()
    io["pad_bias"] = nc.dram_tensor(
        "pad_bias", [128, B, NJT], F32, kind="ExternalInput"
    ).ap()
    # causal masks for the 4 diagonal offsets, duplicated along the head pair:
    # [128, 4, 2*512]
    io["cmask"] = nc.dram_tensor(
        "cmask", [128, 4, 2 * 512], F16, kind="ExternalInput"
    ).ap()
    io["y"] = nc.dram_tensor("y_partial", [BS, D], F16, kind="ExternalOutput").ap()
    # scratch for the softmax-denominator broadcast (DRAM allows 0-step APs)
    io["rscratch"] = nc.dram_tensor("rscratch", [B * 4, H_LOC * 512], F16).ap()

    with tile.TileContext(nc) as tc, ExitStack() as ctx:
        pools = {
            "const": ctx.enter_context(tc.tile_pool(name="const", bufs=1)),
            "xpool": ctx.enter_context(tc.tile_pool(name="xpool", bufs=44)),
            "ppool": ctx.enter_context(tc.tile_pool(name="ppool", bufs=6)),
            "mpool": ctx.enter_context(tc.tile_pool(name="mpool", bufs=2)),
            "ypool": ctx.enter_context(tc.tile_pool(name="ypool", bufs=3)),
            # PSUM budget (8 banks): ps 2x2 + po 1x2 + mm 2x1 = 8
            "psum_s": ctx.enter_context(
                tc.tile_pool(name="psum_s", bufs=2, space="PSUM")
            ),
            "psum_o": ctx.enter_context(
                tc.tile_pool(name="psum_o", bufs=1, space="PSUM")
            ),
            "psum_mm": ctx.enter_context(
                tc.tile_pool(name="psum_mm", bufs=2, space="PSUM")
            ),
        }
        _program(pools, tc, io, reps)
    nc.compile()
    return nc


def _program(pools, tc, io, reps):
    nc = tc.nc

    const = pools["const"]
    xpool = pools["xpool"]
    ppool = pools["ppool"]
    mpool = pools["mpool"]
    ypool = pools["ypool"]
    psum_s = pools["psum_s"]
    psum_o = pools["psum_o"]
    psum_mm = pools["psum_mm"]

    # ---- constants / persistent buffers (loaded once for all reps) ----
    w_sb = {}
    for nm in ("wq", "wk", "wv"):
        t = const.tile([128, 8, 128], F16, name=f"{nm}_sb")
        nc.sync.dma_start(out=t, in_=io[nm])
        w_sb[nm] = t
    woT_sb = const.tile([128, D], F16, name="woT_sb")
    nc.sync.dma_start(out=woT_sb, in_=io["woT"])
    cmask_sb = const.tile([128, 4, 2 * 512], F16, name="cmask_sb")
    nc.sync.dma_start(out=cmask_sb, in_=io["cmask"])
    pbias_sb = const.tile([128, B, NJT], F32, name="pbias_sb")
    nc.sync.dma_start(out=pbias_sb, in_=io["pad_bias"])

    qhT_sb = const.tile([128, BS], F16, name="qhT_sb")  # [c2, b*S + i]
    khT_sb = const.tile([128, BS], F16, name="khT_sb")
    vaug_sb = const.tile([128, B, H_LOC, NJT, DK + 1], F16, name="vaug_sb")
    nc.vector.memset(vaug_sb[:, :, :, :, DK : DK + 1], 1.0)
    aT_sb = const.tile([128, BS], F16, name="aT_sb")
    araw = {
        b: const.tile([DK + 1, H_LOC, S], F16, name=f"araw_{b}") for b in range(B)
    }

    # ---- phase A as a list of work units (closures) for feeding into B ----
    def a_units(ss, rep):
        xts = {}
        units = []

        def load_unit():
            for nm in ("q", "k", "v"):
                xT = io["x" + nm + "T"]
                for kk in range(8):
                    xt = xpool.tile(
                        [128, 1024], F16, name=f"x{nm}_{rep}_{ss}_{kk}", tag="xt"
                    )
                    nc.sync.dma_start(
                        out=xt,
                        in_=xT[
                            kk * 128 : (kk + 1) * 128, ss * 1024 : (ss + 1) * 1024
                        ],
                    )
                    xts[nm, kk] = xt

        units.append(load_unit)

        def qk_unit(nm, sc):
            def run():
                ps = psum_mm.tile(
                    [128, 512], F32, name=f"psA{nm}_{rep}_{ss}_{sc}", tag="mm"
                )
                for kk in range(8):
                    nc.tensor.matmul(
                        ps,
                        lhsT=w_sb["w" + nm][:, kk, :],
                        rhs=xts[nm, kk][:, sc * 512 : (sc + 1) * 512],
                        start=(kk == 0),
                        stop=(kk == 7),
                    )
                col = ss * 1024 + sc * 512
                outbuf = {"q": qhT_sb, "k": khT_sb}[nm]
                nc.vector.tensor_copy(outbuf[:, col : col + 512], ps)

            return run

        for nm in ("q", "k"):
            for sc in range(2):
                units.append(qk_unit(nm, sc))

        vps = {}

        def v_unit(sp, half):
            def run():
                if half == 0:
                    vps[sp] = psum_mm.tile(
                        [128, 4, 128], F32, name=f"psV_{rep}_{ss}_{sp}", tag="mm"
                    )
                ps = vps[sp]
                for i4 in (2 * half, 2 * half + 1):
                    sl = sp * 4 + i4
                    for kk in range(8):
                        nc.tensor.matmul(
                            ps[:, i4, :],
                            lhsT=xts["v", kk][:, sl * 128 : (sl + 1) * 128],
                            rhs=w_sb["wv"][:, kk, :],
                            start=(kk == 0),
                            stop=(kk == 7),
                        )
                if half == 1:
                    sch0 = ss * 8 + sp * 4
                    b, jt0 = divmod(sch0, NJT)
                    nc.vector.tensor_copy(
                        vaug_sb[:, b, :, jt0 : jt0 + 4, 0:DK].rearrange(
                            "p h j c -> p j h c"
                        ),
                        ps.rearrange("p j (h c) -> p j h c", h=H_LOC),
                    )

            return run

        for sp in range(2):
            for half in range(2):
                units.append(v_unit(sp, half))
        return units

    # ---- divide: split so the DRAM round-trip latency of the denominator
    # broadcast never blocks the in-order DVE/Sync queues: `pre` (Pool DMAs)
    # right after the araw copy, `post` (DVE) a few jt-iterations later.
    def divide_pre(b, ic, rep):
        dchunk = slice(ic * 512, (ic + 1) * 512)
        rrow = io["rscratch"][b * 4 + ic : b * 4 + ic + 1, :]
        nc.gpsimd.dma_start(out=rrow, in_=araw[b][DK : DK + 1, :, dchunk])
        # fp16 DRAM row -> fp16 SBUF with partition replication (casting in
        # a swdge DMA is software-slow; upcast on DVE in divide_post instead)
        den = mpool.tile([DK, H_LOC, 512], F16, name=f"den_{rep}{b}{ic}", tag="den")
        dbcast = bass.AP(
            tensor=rrow.tensor,
            offset=rrow.offset,
            ap=[[0, DK], [512, H_LOC], [1, 512]],
        )
        nc.gpsimd.dma_start(out=den, in_=dbcast)
        return den

    def divide_post(b, ic, den, rep):
        dchunk = slice(ic * 512, (ic + 1) * 512)
        dcols = slice(b * S + ic * 512, b * S + (ic + 1) * 512)
        den32 = mpool.tile([DK, H_LOC, 512], F32, name=f"d32_{rep}{b}{ic}", tag="d32")
        nc.vector.tensor_copy(den32, den)
        denr = mpool.tile([DK, H_LOC, 512], F32, name=f"dr_{rep}{b}{ic}", tag="denr")
        nc.vector.reciprocal_approx_fast(denr, den32)
        nc.vector.tensor_mul(
            aT_sb[0:DK, dcols], araw[b][0:DK, 0, dchunk], denr[:, 0, :]
        )
        tmpa = mpool.tile([DK, 512], F16, name=f"ta_{rep}{b}{ic}", tag="ta")
        nc.vector.tensor_mul(tmpa, araw[b][0:DK, 1, dchunk], denr[:, 1, :])
        # partition remap 0:64 -> 64:128 via SBUF->SBUF DMA (Pool queue)
        nc.gpsimd.dma_start(out=aT_sb[DK : 2 * DK, dcols], in_=tmpa)

    def phase_c(b, schs, rep, tail=False):
        for sch in schs:
            ysb = ypool.tile([128, 1024], F16, name=f"ysb_{rep}{b}{sch}", tag="ysb")
            for eh in range(2):
                py = psum_mm.tile(
                    [128, 512], F32, name=f"psC_{rep}{b}{sch}{eh}", tag="mm"
                )
                nc.tensor.matmul(
                    py,
                    lhsT=aT_sb[:, b * S + sch * 128 : b * S + (sch + 1) * 128],
                    rhs=woT_sb[:, eh * 512 : (eh + 1) * 512],
                    start=True,
                    stop=True,
                )
                if tail and eh == 0:
                    nc.scalar.copy(ysb[:, 0:512], py)
                else:
                    nc.vector.tensor_copy(ysb[:, eh * 512 : (eh + 1) * 512], py)
            r0 = b * S + sch * 128
            nc.gpsimd.dma_start(out=io["y"][r0 : r0 + 128, :], in_=ysb)

    # ---- phase B: attention (head pair together) + chunked division + C ----
    # `post_in` is the previous phase's final (b, ic, den) divide_post, run at
    # slot 2 here so its DRAM round-trip never blocks the DVE queue. Own
    # divide_posts run ~5 slots after their pre. `c_feed` is a list of
    # (min_slot, b, sch) output tiles, emitted once their aT chunk is ready.
    # `feed` holds phase-A units, one per slot, front-loaded.
    POST_K = {0: 8, 1: 16, 2: 28}  # slot for divide_post of ic 0..2

    def phase_b(b, rep, post_in=None, c_feed=(), feed=()):
        feed = list(feed)
        c_items = sorted(c_feed)
        pending = {}  # ic -> den
        k = 0
        for ic in range(4):  # query chunks of 512
            njt = 4 * (ic + 1)  # causal: keys up to end of this query chunk
            ichunk = slice(ic * 512, (ic + 1) * 512)
            po = psum_o.tile(
                [DK + 1, H_LOC, 512], F32, name=f"po_{rep}{b}{ic}", tag="po"
            )
            for jt in range(njt):
                if post_in is not None and k == 2:
                    b_p, ic_p, den = post_in
                    divide_post(b_p, ic_p, den, rep)
                    post_in = None
                for ic_p, kk in list(POST_K.items()):
                    if k == kk and ic_p in pending:
                        divide_post(b, ic_p, pending.pop(ic_p), rep)
                # one PE-filler event per slot: a due C tile, else an A unit
                if c_items and c_items[0][0] <= k:
                    _, b_c, sch = c_items.pop(0)
                    phase_c(b_c, [sch], rep)
                elif feed:
                    feed.pop(0)()
                k += 1
                jcols = slice(b * S + jt * 128, b * S + (jt + 1) * 128)
                # diagonal tiles: columns i < (jt-4ic)*128 are fully masked --
                # skip them in the scores matmul, exp, mask, and PV.
                o = jt - 4 * ic
                lo = o * 128 if o >= 0 else 0
                ps = psum_s.tile(
                    [128, H_LOC, 512], F32, name=f"psS_{rep}{b}{ic}{jt}", tag="ps"
                )
                for h in range(H_LOC):
                    r0 = DK * h
                    nc.tensor.matmul(
                        ps[:, h, lo:512],
                        lhsT=khT_sb[r0 : r0 + DK, jcols],
                        rhs=qhT_sb[
                            r0 : r0 + DK,
                            b * S + ic * 512 + lo : b * S + (ic + 1) * 512,
                        ],
                        start=True,
                        stop=True,
                    )
                pe = ppool.tile(
                    [128, H_LOC, 512], F16, name=f"pe_{rep}{b}{ic}{jt}", tag="pe"
                )
                nc.scalar.activation(
                    pe[:, :, lo:512],
                    ps[:, :, lo:512],
                    FT.Exp,
                    bias=pbias_sb[:, b, jt : jt + 1],
                    scale=0.125,
                )
                if o >= 0:  # diagonal tile: causal zero-mask on the live slice
                    cmv = cmask_sb[:, o, :].rearrange("p (h i) -> p h i", h=H_LOC)
                    nc.vector.tensor_mul(
                        pe[:, :, lo:512], pe[:, :, lo:512], cmv[:, :, lo:512]
                    )
                for h in range(H_LOC):
                    nc.tensor.matmul(
                        po[:, h, lo:512],
                        lhsT=vaug_sb[:, b, h, jt, :],
                        rhs=pe[:, h, lo:512],
                        start=(jt == 0),
                        stop=(jt == njt - 1),
                    )
            # one quick copy releases po
            nc.scalar.copy(araw[b][:, :, ichunk], po)
            pending[ic] = divide_pre(b, ic, rep)
        for u in feed:  # leftovers (feed longer than jt count)
            u()
        for _, b_c, sch in c_items:  # leftover C tiles
            phase_c(b_c, [sch], rep)
        return (b, 3, pending.pop(3))

    # ---- main schedule: B phases host A (and C) work units ----
    # Steady state per rep: B0 hosts A(ss2)+A(ss3) + C(b1 prev-rep, 12:16) +
    # C(b0, 0:12); B1 hosts next rep's A(ss0)+A(ss1) + C(b0, 12:16) +
    # C(b1, 0:12). Each phase's final divide_post is carried into the next
    # phase so its DRAM round-trip latency is hidden.
    def c_sched(b_new, b_old):
        # own-b tiles become ready ~5 slots after their divide_post (the
        # post->recip->muls->remap chain is ~4us); stagger to avoid DVE bursts
        items = [
            (POST_K[sch // 4] + 5 + 2 * (sch % 4), b_new, sch) for sch in range(12)
        ]
        # other-b ic3 tiles: ready a few slots after the carried post (slot 2)
        items += [(7 + 2 * (sch - 12), b_old, sch) for sch in range(12, 16)]
        return items

    post = None
    for r in range(reps):
        if r == 0:
            for u in a_units(0, 0) + a_units(1, 0):
                u()
        cf0 = c_sched(0, 1) if r > 0 else [
            (POST_K[sch // 4] + 5 + 2 * (sch % 4), 0, sch) for sch in range(12)
        ]
        post = phase_b(0, r, post_in=post, c_feed=cf0,
                       feed=a_units(2, r) + a_units(3, r))
        nxt = a_units(0, r + 1) + a_units(1, r + 1) if r + 1 < reps else []
        post = phase_b(1, r, post_in=post, c_feed=c_sched(1, 0), feed=nxt)
    # tail: final divide_post + last output tiles
    b_p, ic_p, den = post
    divide_post(b_p, ic_p, den, reps)
    phase_c(1, range(12, 16), reps, tail=True)


def get_nc():
    if "nc" not in _CACHE:
        _CACHE["nc"] = _build_nc()
    return _CACHE["nc"]


def prep_inputs(q, k, v, mask, Wq, Wk, Wv, Wo):
    """Host-side shard prep: transposes, fp16 casts, per-core weight slices."""
    q = np.asarray(q, dtype=np.float32).reshape(BS, D)
    k = np.asarray(k, dtype=np.float32).reshape(BS, D)
    v = np.asarray(v, dtype=np.float32).reshape(BS, D)
    mask = np.asarray(mask)
    Wq, Wk, Wv, Wo = (np.asarray(w, dtype=np.float32) for w in (Wq, Wk, Wv, Wo))

    xqT = np.ascontiguousarray(q.T).astype(np.float16)
    xkT = np.ascontiguousarray(k.T).astype(np.float16)
    xvT = np.ascontiguousarray(v.T).astype(np.float16)

    pb = np.where(mask == 0, np.float32(-1e9), np.float32(0.0)).astype(np.float32)
    # [B, S] -> [128, B, S//128]  (partition = j % 128, col = key tile)
    pad_bias = np.ascontiguousarray(pb.reshape(B, S // 128, 128).transpose(2, 0, 1))

    p_idx = np.arange(128)[:, None, None]
    o_idx = np.arange(4)[None, :, None]
    q_idx = np.arange(512)[None, None, :]
    cm = (q_idx >= o_idx * 128 + p_idx).astype(np.float16)  # [128, 4, 512]
    cmask = np.ascontiguousarray(
        np.broadcast_to(cm[:, :, None, :], (128, 4, H_LOC, 512)).reshape(
            128, 4, H_LOC * 512
        )
    )

    def wslice(Wmat, c):
        ws = Wmat[c * M_LOC : (c + 1) * M_LOC, :]  # [128 out, 1024 in]
        # -> [p(=d%128), kk(=d//128), c2]
        return np.ascontiguousarray(
            ws.T.reshape(8, 128, M_LOC).transpose(1, 0, 2)
        ).astype(np.float16)

    in_maps = []
    for c in range(N_CORES):
        woT_c = np.ascontiguousarray(Wo[:, c * M_LOC : (c + 1) * M_LOC].T).astype(
            np.float16
        )
        in_maps.append(
            {
                "xqT": xqT,
                "xkT": xkT,
                "xvT": xvT,
                "wq": wslice(Wq, c),
                "wk": wslice(Wk, c),
                "wv": wslice(Wv, c),
                "woT": woT_c,
                "pad_bias": pad_bias,
                "cmask": cmask,
            }
        )
    return in_maps


def gather_output(results):
    acc = np.zeros((BS, D), dtype=np.float32)
    for r in results:
        acc += r["y_partial"].astype(np.float32)
    return acc.reshape(B, S, D)


def kernel(q, k, v, mask, Wq, Wk, Wv, Wo):
    nc = get_nc()
    in_maps = prep_inputs(q, k, v, mask, Wq, Wk, Wv, Wo)
    res = run_bass_kernel_spmd(nc, in_maps, core_ids=list(range(N_CORES)))
    return gather_output(res.results)


# revision 43
# speedup vs baseline: 2.6279x; 1.0375x over previous
"""Multi-head causal attention (B=2, S=2048, D=1024, H=16) on 8 trn2 NeuronCores.

Sharding: 2 heads per core (tensor-parallel over the 16 heads). Each core
receives the full (host-pre-transposed, fp16) activations plus its own slice
of the projection weights, computes

    qhT/khT = (Wq_l @ x.T)      [128, B*S]   (head dim on partitions)
    vh      = x @ Wv_l.T        per 128-row chunk, stored [j, c] + ones column
    S_T     = kh @ qh.T / 8     per (b, h-pair), keys on partitions
    P       = exp(S_T + pad_bias) * causal_mask
    attT    = (v_aug.T @ P) -> rows 0:64 = att.T, row 64 = softmax denominator
    aT      = attT / denom      (denominator broadcast via DMA replication)
    y_part  = A_local @ Wo[:, mslice].T     [B*S, D] partial sums

The host sums the 8 partial outputs (standard row-parallel unshard) and
reshapes to [B, S, D] float32.

Schedule: the exp-bound attention loops (phase B) are fed with phase-A
projection work units so the tensor engine fills its exp-wait bubbles:
B(b=0) hosts A(ss2)+A(ss3); B(b=1) hosts the NEXT repetition's A(ss0)+A(ss1)
plus the output-projection (phase C) tiles. DMA queues: Sync streams x tiles
only; the divide round-trip and y writes ride the idle Pool (gpsimd) queue.
"""

import os
import sys

import numpy as np

try:
    import concourse.bass as bass
except ImportError:  # fallback if sitecustomize did not add the repo
    for _p in ("/opt/trn_rl_repo", "/root/.axon_site/_ro/trn_rl_repo"):
        if os.path.isdir(_p) and _p not in sys.path:
            sys.path.insert(0, _p)
    import concourse.bass as bass  # noqa: F401

import concourse.tile as tile
from concourse import bacc, mybir
from concourse.bass_utils import run_bass_kernel_spmd

B, S, D, H, DK = 2, 2048, 1024, 16, 64
BS = B * S                # 4096
N_CORES = 8
H_LOC = H // N_CORES      # 2 heads per core
M_LOC = H_LOC * DK        # 128 local concat dim
NJT = S // 128            # 16 key tiles per batch

F16 = mybir.dt.float16
F32 = mybir.dt.float32
FT = mybir.ActivationFunctionType

_CACHE = {}


def _build_nc(reps=1):
    """Build + compile the per-core Bass program (identical across cores).

    reps > 1 repeats the whole (idempotent) body back-to-back inside one
    NEFF — used by test.py to time one rep via the slope between NEFFs.
    """
    from contextlib import ExitStack

    nc = bacc.Bacc(
        "TRN2", target_bir_lowering=False, debug=False, enable_asserts=False
    )

    io = {}
    for nm in ("xqT", "xkT", "xvT"):
        io[nm] = nc.dram_tensor(nm, [D, BS], F16, kind="ExternalInput").ap()
    for nm in ("wq", "wk", "wv"):
        io[nm] = nc.dram_tensor(nm, [128, 8, 128], F16, kind="ExternalInput").ap()
    io["woT"] = nc.dram_tensor("woT", [128, D], F16, kind="ExternalInput").ap()
    io["pad_bias"] = nc.dram_tensor(
        "pad_bias", [128, B, NJT], F32, kind="ExternalInput"
    ).ap()
    # causal masks for the 4 diagonal offsets, duplicated along the head pair:
    # [128, 4, 2*512]
    io["cmask"] = nc.dram_tensor(
        "cmask", [128, 4, 2 * 512], F16, kind="ExternalInput"
    ).ap()
    io["y"] = nc.dram_tensor("y_partial", [BS, D], F16, kind="ExternalOutput").ap()
    # scratch for the softmax-denominator broadcast (DRAM allows 0-step APs)
    io["rscratch"] = nc.dram_tensor("rscratch", [B * 4, H_LOC * 512], F16).ap()

    with tile.TileContext(nc) as tc, ExitStack() as ctx:
        pools = {
            "const": ctx.enter_context(tc.tile_pool(name="const", bufs=1)),
            "xpool": ctx.enter_context(tc.tile_pool(name="xpool", bufs=44)),
            "ppool": ctx.enter_context(tc.tile_pool(name="ppool", bufs=6)),
            "mpool": ctx.enter_context(tc.tile_pool(name="mpool", bufs=2)),
            "ypool": ctx.enter_context(tc.tile_pool(name="ypool", bufs=3)),
            # PSUM budget (8 banks): ps 2x2 + po 1x2 + mm 2x1 = 8
            "psum_s": ctx.enter_context(
                tc.tile_pool(name="psum_s", bufs=2, space="PSUM")
            ),
            "psum_o": ctx.enter_context(
                tc.tile_pool(name="psum_o", bufs=1, space="PSUM")
            ),
            "psum_mm": ctx.enter_context(
                tc.tile_pool(name="psum_mm", bufs=2, space="PSUM")
            ),
        }
        _program(pools, tc, io, reps)
    nc.compile()
    return nc


def _program(pools, tc, io, reps):
    nc = tc.nc

    const = pools["const"]
    xpool = pools["xpool"]
    ppool = pools["ppool"]
    mpool = pools["mpool"]
    ypool = pools["ypool"]
    psum_s = pools["psum_s"]
    psum_o = pools["psum_o"]
    psum_mm = pools["psum_mm"]

    # ---- constants / persistent buffers (loaded once for all reps) ----
    w_sb = {}
    for nm in ("wq", "wk", "wv"):
        t = const.tile([128, 8, 128], F16, name=f"{nm}_sb")
        nc.sync.dma_start(out=t, in_=io[nm])
        w_sb[nm] = t
    woT_sb = const.tile([128, D], F16, name="woT_sb")
    nc.sync.dma_start(out=woT_sb, in_=io["woT"])
    cmask_sb = const.tile([128, 4, 2 * 512], F16, name="cmask_sb")
    nc.sync.dma_start(out=cmask_sb, in_=io["cmask"])
    pbias_sb = const.tile([128, B, NJT], F32, name="pbias_sb")
    nc.sync.dma_start(out=pbias_sb, in_=io["pad_bias"])

    qhT_sb = const.tile([128, BS], F16, name="qhT_sb")  # [c2, b*S + i]
    khT_sb = const.tile([128, BS], F16, name="khT_sb")
    vaug_sb = const.tile([128, B, H_LOC, NJT, DK + 1], F16, name="vaug_sb")
    nc.vector.memset(vaug_sb[:, :, :, :, DK : DK + 1], 1.0)
    aT_sb = const.tile([128, BS], F16, name="aT_sb")
    araw = {
        b: const.tile([DK + 1, H_LOC, S], F16, name=f"araw_{b}") for b in range(B)
    }

    # ---- phase A as a list of work units (closures) for feeding into B ----
    def a_units(ss, rep):
        xts = {}
        units = []

        def load_unit():
            for nm in ("q", "k", "v"):
                xT = io["x" + nm + "T"]
                for kk in range(8):
                    xt = xpool.tile(
                        [128, 1024], F16, name=f"x{nm}_{rep}_{ss}_{kk}", tag="xt"
                    )
                    nc.sync.dma_start(
                        out=xt,
                        in_=xT[
                            kk * 128 : (kk + 1) * 128, ss * 1024 : (ss + 1) * 1024
                        ],
                    )
                    xts[nm, kk] = xt

        units.append(load_unit)

        def qk_unit(nm, sc):
            def run():
                ps = psum_mm.tile(
                    [128, 512], F32, name=f"psA{nm}_{rep}_{ss}_{sc}", tag="mm"
                )
                for kk in range(8):
                    nc.tensor.matmul(
                        ps,
                        lhsT=w_sb["w" + nm][:, kk, :],
                        rhs=xts[nm, kk][:, sc * 512 : (sc + 1) * 512],
                        start=(kk == 0),
                        stop=(kk == 7),
                    )
                col = ss * 1024 + sc * 512
                outbuf = {"q": qhT_sb, "k": khT_sb}[nm]
                nc.vector.tensor_copy(outbuf[:, col : col + 512], ps)

            return run

        for nm in ("q", "k"):
            for sc in range(2):
                units.append(qk_unit(nm, sc))

        vps = {}

        def v_unit(sp, half):
            def run():
                if half == 0:
                    vps[sp] = psum_mm.tile(
                        [128, 4, 128], F32, name=f"psV_{rep}_{ss}_{sp}", tag="mm"
                    )
                ps = vps[sp]
                for i4 in (2 * half, 2 * half + 1):
                    sl = sp * 4 + i4
                    for kk in range(8):
                        nc.tensor.matmul(
                            ps[:, i4, :],
                            lhsT=xts["v", kk][:, sl * 128 : (sl + 1) * 128],
                            rhs=w_sb["wv"][:, kk, :],
                            start=(kk == 0),
                            stop=(kk == 7),
                        )
                if half == 1:
                    sch0 = ss * 8 + sp * 4
                    b, jt0 = divmod(sch0, NJT)
                    nc.vector.tensor_copy(
                        vaug_sb[:, b, :, jt0 : jt0 + 4, 0:DK].rearrange(
                            "p h j c -> p j h c"
                        ),
                        ps.rearrange("p j (h c) -> p j h c", h=H_LOC),
                    )

            return run

        for sp in range(2):
            for half in range(2):
                units.append(v_unit(sp, half))
        return units

    # ---- divide: split so the DRAM round-trip latency of the denominator
    # broadcast never blocks the in-order DVE/Sync queues: `pre` (Pool DMAs)
    # right after the araw copy, `post` (DVE) a few jt-iterations later.
    def divide_pre(b, ic, rep):
        dchunk = slice(ic * 512, (ic + 1) * 512)
        rrow = io["rscratch"][b * 4 + ic : b * 4 + ic + 1, :]
        nc.gpsimd.dma_start(out=rrow, in_=araw[b][DK : DK + 1, :, dchunk])
        # fp16 DRAM row -> fp16 SBUF with partition replication (casting in
        # a swdge DMA is software-slow; upcast on DVE in divide_post instead)
        den = mpool.tile([DK, H_LOC, 512], F16, name=f"den_{rep}{b}{ic}", tag="den")
        dbcast = bass.AP(
            tensor=rrow.tensor,
            offset=rrow.offset,
            ap=[[0, DK], [512, H_LOC], [1, 512]],
        )
        nc.gpsimd.dma_start(out=den, in_=dbcast)
        return den

    def divide_post(b, ic, den, rep):
        dchunk = slice(ic * 512, (ic + 1) * 512)
        dcols = slice(b * S + ic * 512, b * S + (ic + 1) * 512)
        den32 = mpool.tile([DK, H_LOC, 512], F32, name=f"d32_{rep}{b}{ic}", tag="d32")
        nc.vector.tensor_copy(den32, den)
        denr = mpool.tile([DK, H_LOC, 512], F32, name=f"dr_{rep}{b}{ic}", tag="denr")
        nc.vector.reciprocal_approx_fast(denr, den32)
        nc.vector.tensor_mul(
            aT_sb[0:DK, dcols], araw[b][0:DK, 0, dchunk], denr[:, 0, :]
        )
        # DVE writes the h1 rows directly at shifted partitions 64:128 — no
        # SBUF->SBUF remap DMA needed (swdge completion costs ~5us).
        nc.vector.tensor_mul(
            aT_sb[DK : 2 * DK, dcols], araw[b][0:DK, 1, dchunk], denr[:, 1, :]
        )

    def phase_c(b, schs, rep, tail=False):
        for sch in schs:
            ysb = ypool.tile([128, 1024], F16, name=f"ysb_{rep}{b}{sch}", tag="ysb")
            for eh in range(2):
                py = psum_mm.tile(
                    [128, 512], F32, name=f"psC_{rep}{b}{sch}{eh}", tag="mm"
                )
                nc.tensor.matmul(
                    py,
                    lhsT=aT_sb[:, b * S + sch * 128 : b * S + (sch + 1) * 128],
                    rhs=woT_sb[:, eh * 512 : (eh + 1) * 512],
                    start=True,
                    stop=True,
                )
                if tail and eh == 0:
                    nc.scalar.copy(ysb[:, 0:512], py)
                else:
                    nc.vector.tensor_copy(ysb[:, eh * 512 : (eh + 1) * 512], py)
            r0 = b * S + sch * 128
            nc.gpsimd.dma_start(out=io["y"][r0 : r0 + 128, :], in_=ysb)

    # ---- phase B: attention (head pair together) + chunked division + C ----
    # `post_in` is the previous phase's final (b, ic, den) divide_post, run at
    # slot 2 here so its DRAM round-trip never blocks the DVE queue. Own
    # divide_posts run ~5 slots after their pre. `c_feed` is a list of
    # (min_slot, b, sch) output tiles, emitted once their aT chunk is ready.
    # `feed` holds phase-A units, one per slot, front-loaded.
    POST_K = {0: 8, 1: 16, 2: 28}  # slot for divide_post of ic 0..2

    def phase_b(b, rep, post_in=None, c_feed=(), feed=()):
        feed = list(feed)
        c_items = sorted(c_feed)
        pending = {}  # ic -> den
        k = 0
        for ic in range(4):  # query chunks of 512
            njt = 4 * (ic + 1)  # causal: keys up to end of this query chunk
            ichunk = slice(ic * 512, (ic + 1) * 512)
            po = psum_o.tile(
                [DK + 1, H_LOC, 512], F32, name=f"po_{rep}{b}{ic}", tag="po"
            )
            for jt in range(njt):
                if post_in is not None and k == 2:
                    b_p, ic_p, den = post_in
                    divide_post(b_p, ic_p, den, rep)
                    post_in = None
                for ic_p, kk in list(POST_K.items()):
                    if k == kk and ic_p in pending:
                        divide_post(b, ic_p, pending.pop(ic_p), rep)
                # one PE-filler event per slot: a due C tile, else an A unit
                if c_items and c_items[0][0] <= k:
                    _, b_c, sch = c_items.pop(0)
                    phase_c(b_c, [sch], rep)
                elif feed:
                    feed.pop(0)()
                k += 1
                jcols = slice(b * S + jt * 128, b * S + (jt + 1) * 128)
                # diagonal tiles: columns i < (jt-4ic)*128 are fully masked --
                # skip them in the scores matmul, exp, mask, and PV.
                o = jt - 4 * ic
                lo = o * 128 if o >= 0 else 0
                ps = psum_s.tile(
                    [128, H_LOC, 512], F32, name=f"psS_{rep}{b}{ic}{jt}", tag="ps"
                )
                for h in range(H_LOC):
                    r0 = DK * h
                    nc.tensor.matmul(
                        ps[:, h, lo:512],
                        lhsT=khT_sb[r0 : r0 + DK, jcols],
                        rhs=qhT_sb[
                            r0 : r0 + DK,
                            b * S + ic * 512 + lo : b * S + (ic + 1) * 512,
                        ],
                        start=True,
                        stop=True,
                    )
                pe = ppool.tile(
                    [128, H_LOC, 512], F16, name=f"pe_{rep}{b}{ic}{jt}", tag="pe"
                )
                nc.scalar.activation(
                    pe[:, :, lo:512],
                    ps[:, :, lo:512],
                    FT.Exp,
                    bias=pbias_sb[:, b, jt : jt + 1],
                    scale=0.125,
                )
                if o >= 0:  # diagonal tile: causal zero-mask on the live slice
                    cmv = cmask_sb[:, o, :].rearrange("p (h i) -> p h i", h=H_LOC)
                    nc.vector.tensor_mul(
                        pe[:, :, lo:512], pe[:, :, lo:512], cmv[:, :, lo:512]
                    )
                for h in range(H_LOC):
                    nc.tensor.matmul(
                        po[:, h, lo:512],
                        lhsT=vaug_sb[:, b, h, jt, :],
                        rhs=pe[:, h, lo:512],
                        start=(jt == 0),
                        stop=(jt == njt - 1),
                    )
            # one quick copy releases po
            nc.scalar.copy(araw[b][:, :, ichunk], po)
            pending[ic] = divide_pre(b, ic, rep)
        for u in feed:  # leftovers (feed longer than jt count)
            u()
        for _, b_c, sch in c_items:  # leftover C tiles
            phase_c(b_c, [sch], rep)
        return (b, 3, pending.pop(3))

    # ---- main schedule: B phases host A (and C) work units ----
    # Steady state per rep: B0 hosts A(ss2)+A(ss3) + C(b1 prev-rep, 12:16) +
    # C(b0, 0:12); B1 hosts next rep's A(ss0)+A(ss1) + C(b0, 12:16) +
    # C(b1, 0:12). Each phase's final divide_post is carried into the next
    # phase so its DRAM round-trip latency is hidden.
    def c_sched(b_new, b_old):
        # own-b tiles become ready ~5 slots after their divide_post (the
        # post->recip->muls->remap chain is ~4us); stagger to avoid DVE bursts
        items = [
            (POST_K[sch // 4] + 5 + 2 * (sch % 4), b_new, sch) for sch in range(12)
        ]
        # other-b ic3 tiles: ready a few slots after the carried post (slot 2)
        items += [(7 + 2 * (sch - 12), b_old, sch) for sch in range(12, 16)]
        return items

    post = None
    for r in range(reps):
        if r == 0:
            for u in a_units(0, 0) + a_units(1, 0):
                u()
        cf0 = c_sched(0, 1) if r > 0 else [
            (POST_K[sch // 4] + 5 + 2 * (sch % 4), 0, sch) for sch in range(12)
        ]
        post = phase_b(0, r, post_in=post, c_feed=cf0,
                       feed=a_units(2, r) + a_units(3, r))
        nxt = a_units(0, r + 1) + a_units(1, r + 1) if r + 1 < reps else []
        post = phase_b(1, r, post_in=post, c_feed=c_sched(1, 0), feed=nxt)
    # tail: final divide_post + last output tiles
    b_p, ic_p, den = post
    divide_post(b_p, ic_p, den, reps)
    phase_c(1, range(12, 16), reps, tail=True)


def get_nc():
    if "nc" not in _CACHE:
        _CACHE["nc"] = _build_nc()
    return _CACHE["nc"]


def prep_inputs(q, k, v, mask, Wq, Wk, Wv, Wo):
    """Host-side shard prep: transposes, fp16 casts, per-core weight slices."""
    q = np.asarray(q, dtype=np.float32).reshape(BS, D)
    k = np.asarray(k, dtype=np.float32).reshape(BS, D)
    v = np.asarray(v, dtype=np.float32).reshape(BS, D)
    mask = np.asarray(mask)
    Wq, Wk, Wv, Wo = (np.asarray(w, dtype=np.float32) for w in (Wq, Wk, Wv, Wo))

    xqT = np.ascontiguousarray(q.T).astype(np.float16)
    xkT = np.ascontiguousarray(k.T).astype(np.float16)
    xvT = np.ascontiguousarray(v.T).astype(np.float16)

    pb = np.where(mask == 0, np.float32(-1e9), np.float32(0.0)).astype(np.float32)
    # [B, S] -> [128, B, S//128]  (partition = j % 128, col = key tile)
    pad_bias = np.ascontiguousarray(pb.reshape(B, S // 128, 128).transpose(2, 0, 1))

    p_idx = np.arange(128)[:, None, None]
    o_idx = np.arange(4)[None, :, None]
    q_idx = np.arange(512)[None, None, :]
    cm = (q_idx >= o_idx * 128 + p_idx).astype(np.float16)  # [128, 4, 512]
    cmask = np.ascontiguousarray(
        np.broadcast_to(cm[:, :, None, :], (128, 4, H_LOC, 512)).reshape(
            128, 4, H_LOC * 512
        )
    )

    def wslice(Wmat, c):
        ws = Wmat[c * M_LOC : (c + 1) * M_LOC, :]  # [128 out, 1024 in]
        # -> [p(=d%128), kk(=d//128), c2]
        return np.ascontiguousarray(
            ws.T.reshape(8, 128, M_LOC).transpose(1, 0, 2)
        ).astype(np.float16)

    in_maps = []
    for c in range(N_CORES):
        woT_c = np.ascontiguousarray(Wo[:, c * M_LOC : (c + 1) * M_LOC].T).astype(
            np.float16
        )
        in_maps.append(
            {
                "xqT": xqT,
                "xkT": xkT,
                "xvT": xvT,
                "wq": wslice(Wq, c),
                "wk": wslice(Wk, c),
                "wv": wslice(Wv, c),
                "woT": woT_c,
                "pad_bias": pad_bias,
                "cmask": cmask,
            }
        )
    return in_maps


def gather_output(results):
    acc = np.zeros((BS, D), dtype=np.float32)
    for r in results:
        acc += r["y_partial"].astype(np.float32)
    return acc.reshape(B, S, D)


def kernel(q, k, v, mask, Wq, Wk, Wv, Wo):
    nc = get_nc()
    in_maps = prep_inputs(q, k, v, mask, Wq, Wk, Wv, Wo)
    res = run_bass_kernel_spmd(nc, in_maps, core_ids=list(range(N_CORES)))
    return gather_output(res.results)


# revision 44
# speedup vs baseline: 2.7016x; 1.0281x over previous
"""Multi-head causal attention (B=2, S=2048, D=1024, H=16) on 8 trn2 NeuronCores.

Sharding: 2 heads per core (tensor-parallel over the 16 heads). Each core
receives the full (host-pre-transposed, fp16) activations plus its own slice
of the projection weights, computes

    qhT/khT = (Wq_l @ x.T)      [128, B*S]   (head dim on partitions)
    vh      = x @ Wv_l.T        per 128-row chunk, stored [j, c] + ones column
    S_T     = kh @ qh.T / 8     per (b, h-pair), keys on partitions
    P       = exp(S_T + pad_bias) * causal_mask
    attT    = (v_aug.T @ P) -> rows 0:64 = att.T, row 64 = softmax denominator
    aT      = attT / denom      (denominator broadcast via DMA replication)
    y_part  = A_local @ Wo[:, mslice].T     [B*S, D] partial sums

The host sums the 8 partial outputs (standard row-parallel unshard) and
reshapes to [B, S, D] float32.

Schedule: the exp-bound attention loops (phase B) are fed with phase-A
projection work units so the tensor engine fills its exp-wait bubbles:
B(b=0) hosts A(ss2)+A(ss3); B(b=1) hosts the NEXT repetition's A(ss0)+A(ss1)
plus the output-projection (phase C) tiles. DMA queues: Sync streams x tiles
only; the divide round-trip and y writes ride the idle Pool (gpsimd) queue.
"""

import os
import sys

import numpy as np

try:
    import concourse.bass as bass
except ImportError:  # fallback if sitecustomize did not add the repo
    for _p in ("/opt/trn_rl_repo", "/root/.axon_site/_ro/trn_rl_repo"):
        if os.path.isdir(_p) and _p not in sys.path:
            sys.path.insert(0, _p)
    import concourse.bass as bass  # noqa: F401

import concourse.tile as tile
from concourse import bacc, mybir
from concourse.bass_utils import run_bass_kernel_spmd

B, S, D, H, DK = 2, 2048, 1024, 16, 64
BS = B * S                # 4096
N_CORES = 8
H_LOC = H // N_CORES      # 2 heads per core
M_LOC = H_LOC * DK        # 128 local concat dim
NJT = S // 128            # 16 key tiles per batch

F16 = mybir.dt.float16
F32 = mybir.dt.float32
FT = mybir.ActivationFunctionType

_CACHE = {}


def _build_nc(reps=1):
    """Build + compile the per-core Bass program (identical across cores).

    reps > 1 repeats the whole (idempotent) body back-to-back inside one
    NEFF — used by test.py to time one rep via the slope between NEFFs.
    """
    from contextlib import ExitStack

    nc = bacc.Bacc(
        "TRN2", target_bir_lowering=False, debug=False, enable_asserts=False
    )

    io = {}
    for nm in ("xqT", "xkT", "xvT"):
        io[nm] = nc.dram_tensor(nm, [D, BS], F16, kind="ExternalInput").ap()
    for nm in ("wq", "wk", "wv"):
        io[nm] = nc.dram_tensor(nm, [128, 8, 128], F16, kind="ExternalInput").ap()
    io["woT"] = nc.dram_tensor("woT", [128, D], F16, kind="ExternalInput").ap()
    io["pad_bias"] = nc.dram_tensor(
        "pad_bias", [128, B, NJT], F32, kind="ExternalInput"
    ).ap()
    # causal masks for the 4 diagonal offsets, duplicated along the head pair:
    # [128, 4, 2*512]
    io["cmask"] = nc.dram_tensor(
        "cmask", [128, 4, 2 * 512], F16, kind="ExternalInput"
    ).ap()
    io["y"] = nc.dram_tensor("y_partial", [BS, D], F16, kind="ExternalOutput").ap()
    # scratch for the softmax-denominator broadcast (DRAM allows 0-step APs)
    io["rscratch"] = nc.dram_tensor("rscratch", [B * 4, H_LOC * 512], F16).ap()

    with tile.TileContext(nc) as tc, ExitStack() as ctx:
        pools = {
            "const": ctx.enter_context(tc.tile_pool(name="const", bufs=1)),
            "xpool": ctx.enter_context(tc.tile_pool(name="xpool", bufs=44)),
            "ppool": ctx.enter_context(tc.tile_pool(name="ppool", bufs=6)),
            "mpool": ctx.enter_context(tc.tile_pool(name="mpool", bufs=2)),
            "ypool": ctx.enter_context(tc.tile_pool(name="ypool", bufs=4)),
            # PSUM budget (8 banks): ps 2x2 + po 1x2 + mm 2x1 = 8
            "psum_s": ctx.enter_context(
                tc.tile_pool(name="psum_s", bufs=2, space="PSUM")
            ),
            "psum_o": ctx.enter_context(
                tc.tile_pool(name="psum_o", bufs=1, space="PSUM")
            ),
            "psum_mm": ctx.enter_context(
                tc.tile_pool(name="psum_mm", bufs=2, space="PSUM")
            ),
        }
        _program(pools, tc, io, reps)
    nc.compile()
    return nc


def _program(pools, tc, io, reps):
    nc = tc.nc

    const = pools["const"]
    xpool = pools["xpool"]
    ppool = pools["ppool"]
    mpool = pools["mpool"]
    ypool = pools["ypool"]
    psum_s = pools["psum_s"]
    psum_o = pools["psum_o"]
    psum_mm = pools["psum_mm"]

    # ---- constants / persistent buffers (loaded once for all reps) ----
    w_sb = {}
    for nm in ("wq", "wk", "wv"):
        t = const.tile([128, 8, 128], F16, name=f"{nm}_sb")
        nc.sync.dma_start(out=t, in_=io[nm])
        w_sb[nm] = t
    woT_sb = const.tile([128, D], F16, name="woT_sb")
    nc.sync.dma_start(out=woT_sb, in_=io["woT"])
    cmask_sb = const.tile([128, 4, 2 * 512], F16, name="cmask_sb")
    nc.sync.dma_start(out=cmask_sb, in_=io["cmask"])
    pbias_sb = const.tile([128, B, NJT], F32, name="pbias_sb")
    nc.sync.dma_start(out=pbias_sb, in_=io["pad_bias"])

    qhT_sb = const.tile([128, BS], F16, name="qhT_sb")  # [c2, b*S + i]
    khT_sb = const.tile([128, BS], F16, name="khT_sb")
    vaug_sb = const.tile([128, B, H_LOC, NJT, DK + 1], F16, name="vaug_sb")
    nc.vector.memset(vaug_sb[:, :, :, :, DK : DK + 1], 1.0)
    aT_sb = const.tile([128, BS], F16, name="aT_sb")
    araw = {
        b: const.tile([DK + 1, H_LOC, S], F16, name=f"araw_{b}") for b in range(B)
    }

    # ---- phase A as a list of work units (closures) for feeding into B ----
    def a_units(ss, rep):
        xts = {}
        units = []

        def load_unit():
            for nm in ("q", "k", "v"):
                xT = io["x" + nm + "T"]
                for kk in range(8):
                    xt = xpool.tile(
                        [128, 1024], F16, name=f"x{nm}_{rep}_{ss}_{kk}", tag="xt"
                    )
                    nc.sync.dma_start(
                        out=xt,
                        in_=xT[
                            kk * 128 : (kk + 1) * 128, ss * 1024 : (ss + 1) * 1024
                        ],
                    )
                    xts[nm, kk] = xt

        units.append(load_unit)

        def qk_unit(nm, sc):
            def run():
                ps = psum_mm.tile(
                    [128, 512], F32, name=f"psA{nm}_{rep}_{ss}_{sc}", tag="mm"
                )
                for kk in range(8):
                    nc.tensor.matmul(
                        ps,
                        lhsT=w_sb["w" + nm][:, kk, :],
                        rhs=xts[nm, kk][:, sc * 512 : (sc + 1) * 512],
                        start=(kk == 0),
                        stop=(kk == 7),
                    )
                col = ss * 1024 + sc * 512
                outbuf = {"q": qhT_sb, "k": khT_sb}[nm]
                nc.vector.tensor_copy(outbuf[:, col : col + 512], ps)

            return run

        for nm in ("q", "k"):
            for sc in range(2):
                units.append(qk_unit(nm, sc))

        vps = {}

        def v_unit(sp, half):
            def run():
                if half == 0:
                    vps[sp] = psum_mm.tile(
                        [128, 4, 128], F32, name=f"psV_{rep}_{ss}_{sp}", tag="mm"
                    )
                ps = vps[sp]
                for i4 in (2 * half, 2 * half + 1):
                    sl = sp * 4 + i4
                    for kk in range(8):
                        nc.tensor.matmul(
                            ps[:, i4, :],
                            lhsT=xts["v", kk][:, sl * 128 : (sl + 1) * 128],
                            rhs=w_sb["wv"][:, kk, :],
                            start=(kk == 0),
                            stop=(kk == 7),
                        )
                if half == 1:
                    sch0 = ss * 8 + sp * 4
                    b, jt0 = divmod(sch0, NJT)
                    nc.vector.tensor_copy(
                        vaug_sb[:, b, :, jt0 : jt0 + 4, 0:DK].rearrange(
                            "p h j c -> p j h c"
                        ),
                        ps.rearrange("p j (h c) -> p j h c", h=H_LOC),
                    )

            return run

        for sp in range(2):
            for half in range(2):
                units.append(v_unit(sp, half))
        return units

    # ---- divide: split so the DRAM round-trip latency of the denominator
    # broadcast never blocks the in-order DVE/Sync queues: `pre` (Pool DMAs)
    # right after the araw copy, `post` (DVE) a few jt-iterations later.
    def divide_pre(b, ic, rep):
        dchunk = slice(ic * 512, (ic + 1) * 512)
        rrow = io["rscratch"][b * 4 + ic : b * 4 + ic + 1, :]
        nc.gpsimd.dma_start(out=rrow, in_=araw[b][DK : DK + 1, :, dchunk])
        # fp16 DRAM row -> fp16 SBUF with partition replication (casting in
        # a swdge DMA is software-slow; upcast on DVE in divide_post instead)
        den = mpool.tile([DK, H_LOC, 512], F16, name=f"den_{rep}{b}{ic}", tag="den")
        dbcast = bass.AP(
            tensor=rrow.tensor,
            offset=rrow.offset,
            ap=[[0, DK], [512, H_LOC], [1, 512]],
        )
        nc.gpsimd.dma_start(out=den, in_=dbcast)
        return den

    def divide_post(b, ic, den, rep):
        dchunk = slice(ic * 512, (ic + 1) * 512)
        dcols = slice(b * S + ic * 512, b * S + (ic + 1) * 512)
        den32 = mpool.tile([DK, H_LOC, 512], F32, name=f"d32_{rep}{b}{ic}", tag="d32")
        nc.vector.tensor_copy(den32, den)
        denr = mpool.tile([DK, H_LOC, 512], F32, name=f"dr_{rep}{b}{ic}", tag="denr")
        nc.vector.reciprocal_approx_fast(denr, den32)
        nc.vector.tensor_mul(
            aT_sb[0:DK, dcols], araw[b][0:DK, 0, dchunk], denr[:, 0, :]
        )
        # DVE writes the h1 rows directly at shifted partitions 64:128 — no
        # SBUF->SBUF remap DMA needed (swdge completion costs ~5us).
        nc.vector.tensor_mul(
            aT_sb[DK : 2 * DK, dcols], araw[b][0:DK, 1, dchunk], denr[:, 1, :]
        )

    def phase_c(b, schs, rep, tail=False):
        for sch in schs:
            ysb = ypool.tile([128, 1024], F16, name=f"ysb_{rep}{b}{sch}", tag="ysb")
            for eh in range(2):
                py = psum_mm.tile(
                    [128, 512], F32, name=f"psC_{rep}{b}{sch}{eh}", tag="mm"
                )
                nc.tensor.matmul(
                    py,
                    lhsT=aT_sb[:, b * S + sch * 128 : b * S + (sch + 1) * 128],
                    rhs=woT_sb[:, eh * 512 : (eh + 1) * 512],
                    start=True,
                    stop=True,
                )
                if tail and eh == 0:
                    nc.scalar.copy(ysb[:, 0:512], py)
                else:
                    nc.vector.tensor_copy(ysb[:, eh * 512 : (eh + 1) * 512], py)
            r0 = b * S + sch * 128
            nc.gpsimd.dma_start(out=io["y"][r0 : r0 + 128, :], in_=ysb)

    # ---- phase B: attention (head pair together) + chunked division + C ----
    # `post_in` is the previous phase's final (b, ic, den) divide_post, run at
    # slot 2 here so its DRAM round-trip never blocks the DVE queue. Own
    # divide_posts run ~5 slots after their pre. `c_feed` is a list of
    # (min_slot, b, sch) output tiles, emitted once their aT chunk is ready.
    # `feed` holds phase-A units, one per slot, front-loaded.
    POST_K = {0: 8, 1: 16, 2: 28}  # slot for divide_post of ic 0..2

    def phase_b(b, rep, post_in=None, c_feed=(), feed=()):
        feed = list(feed)
        c_items = sorted(c_feed)
        pending = {}  # ic -> den
        k = 0
        for ic in range(4):  # query chunks of 512
            njt = 4 * (ic + 1)  # causal: keys up to end of this query chunk
            ichunk = slice(ic * 512, (ic + 1) * 512)
            po = psum_o.tile(
                [DK + 1, H_LOC, 512], F32, name=f"po_{rep}{b}{ic}", tag="po"
            )
            for jt in range(njt):
                if post_in is not None and k == 2:
                    b_p, ic_p, den = post_in
                    divide_post(b_p, ic_p, den, rep)
                    post_in = None
                for ic_p, kk in list(POST_K.items()):
                    if k == kk and ic_p in pending:
                        divide_post(b, ic_p, pending.pop(ic_p), rep)
                # one PE-filler event per slot: a due C tile, else an A unit
                if c_items and c_items[0][0] <= k:
                    _, b_c, sch = c_items.pop(0)
                    phase_c(b_c, [sch], rep)
                elif feed:
                    feed.pop(0)()
                k += 1
                jcols = slice(b * S + jt * 128, b * S + (jt + 1) * 128)
                # diagonal tiles: columns i < (jt-4ic)*128 are fully masked --
                # skip them in the scores matmul, exp, mask, and PV.
                o = jt - 4 * ic
                lo = o * 128 if o >= 0 else 0
                ps = psum_s.tile(
                    [128, H_LOC, 512], F32, name=f"psS_{rep}{b}{ic}{jt}", tag="ps"
                )
                for h in range(H_LOC):
                    r0 = DK * h
                    nc.tensor.matmul(
                        ps[:, h, lo:512],
                        lhsT=khT_sb[r0 : r0 + DK, jcols],
                        rhs=qhT_sb[
                            r0 : r0 + DK,
                            b * S + ic * 512 + lo : b * S + (ic + 1) * 512,
                        ],
                        start=True,
                        stop=True,
                    )
                pe = ppool.tile(
                    [128, H_LOC, 512], F16, name=f"pe_{rep}{b}{ic}{jt}", tag="pe"
                )
                nc.scalar.activation(
                    pe[:, :, lo:512],
                    ps[:, :, lo:512],
                    FT.Exp,
                    bias=pbias_sb[:, b, jt : jt + 1],
                    scale=0.125,
                )
                if o >= 0:  # diagonal tile: causal zero-mask on the live slice
                    cmv = cmask_sb[:, o, :].rearrange("p (h i) -> p h i", h=H_LOC)
                    nc.vector.tensor_mul(
                        pe[:, :, lo:512], pe[:, :, lo:512], cmv[:, :, lo:512]
                    )
                for h in range(H_LOC):
                    nc.tensor.matmul(
                        po[:, h, lo:512],
                        lhsT=vaug_sb[:, b, h, jt, :],
                        rhs=pe[:, h, lo:512],
                        start=(jt == 0),
                        stop=(jt == njt - 1),
                    )
            # one quick copy releases po
            nc.scalar.copy(araw[b][:, :, ichunk], po)
            pending[ic] = divide_pre(b, ic, rep)
        for u in feed:  # leftovers (feed longer than jt count)
            u()
        for _, b_c, sch in c_items:  # leftover C tiles
            phase_c(b_c, [sch], rep)
        return (b, 3, pending.pop(3))

    # ---- main schedule: B phases host A (and C) work units ----
    # Steady state per rep: B0 hosts A(ss2)+A(ss3) + C(b1 prev-rep, 12:16) +
    # C(b0, 0:12); B1 hosts next rep's A(ss0)+A(ss1) + C(b0, 12:16) +
    # C(b1, 0:12). Each phase's final divide_post is carried into the next
    # phase so its DRAM round-trip latency is hidden.
    def c_sched(b_new, b_old):
        # own-b tiles become ready ~5 slots after their divide_post (the
        # post->recip->muls->remap chain is ~4us); stagger to avoid DVE bursts
        items = [
            (POST_K[sch // 4] + 5 + 2 * (sch % 4), b_new, sch) for sch in range(12)
        ]
        # other-b ic3 tiles: ready a few slots after the carried post (slot 2)
        items += [(7 + 2 * (sch - 12), b_old, sch) for sch in range(12, 16)]
        return items

    post = None
    for r in range(reps):
        if r == 0:
            for u in a_units(0, 0) + a_units(1, 0):
                u()
        cf0 = c_sched(0, 1) if r > 0 else [
            (POST_K[sch // 4] + 5 + 2 * (sch % 4), 0, sch) for sch in range(12)
        ]
        post = phase_b(0, r, post_in=post, c_feed=cf0,
                       feed=a_units(2, r) + a_units(3, r))
        nxt = a_units(0, r + 1) + a_units(1, r + 1) if r + 1 < reps else []
        post = phase_b(1, r, post_in=post, c_feed=c_sched(1, 0), feed=nxt)
    # tail: final divide_post + last output tiles
    b_p, ic_p, den = post
    divide_post(b_p, ic_p, den, reps)
    phase_c(1, range(12, 16), reps, tail=True)


def get_nc():
    if "nc" not in _CACHE:
        _CACHE["nc"] = _build_nc()
    return _CACHE["nc"]


def prep_inputs(q, k, v, mask, Wq, Wk, Wv, Wo):
    """Host-side shard prep: transposes, fp16 casts, per-core weight slices."""
    q = np.asarray(q, dtype=np.float32).reshape(BS, D)
    k = np.asarray(k, dtype=np.float32).reshape(BS, D)
    v = np.asarray(v, dtype=np.float32).reshape(BS, D)
    mask = np.asarray(mask)
    Wq, Wk, Wv, Wo = (np.asarray(w, dtype=np.float32) for w in (Wq, Wk, Wv, Wo))

    xqT = np.ascontiguousarray(q.T).astype(np.float16)
    xkT = np.ascontiguousarray(k.T).astype(np.float16)
    xvT = np.ascontiguousarray(v.T).astype(np.float16)

    pb = np.where(mask == 0, np.float32(-1e9), np.float32(0.0)).astype(np.float32)
    # [B, S] -> [128, B, S//128]  (partition = j % 128, col = key tile)
    pad_bias = np.ascontiguousarray(pb.reshape(B, S // 128, 128).transpose(2, 0, 1))

    p_idx = np.arange(128)[:, None, None]
    o_idx = np.arange(4)[None, :, None]
    q_idx = np.arange(512)[None, None, :]
    cm = (q_idx >= o_idx * 128 + p_idx).astype(np.float16)  # [128, 4, 512]
    cmask = np.ascontiguousarray(
        np.broadcast_to(cm[:, :, None, :], (128, 4, H_LOC, 512)).reshape(
            128, 4, H_LOC * 512
        )
    )

    def wslice(Wmat, c):
        ws = Wmat[c * M_LOC : (c + 1) * M_LOC, :]  # [128 out, 1024 in]
        # -> [p(=d%128), kk(=d//128), c2]
        return np.ascontiguousarray(
            ws.T.reshape(8, 128, M_LOC).transpose(1, 0, 2)
        ).astype(np.float16)

    in_maps = []
    for c in range(N_CORES):
        woT_c = np.ascontiguousarray(Wo[:, c * M_LOC : (c + 1) * M_LOC].T).astype(
            np.float16
        )
        in_maps.append(
            {
                "xqT": xqT,
                "xkT": xkT,
                "xvT": xvT,
                "wq": wslice(Wq, c),
                "wk": wslice(Wk, c),
                "wv": wslice(Wv, c),
                "woT": woT_c,
                "pad_bias": pad_bias,
                "cmask": cmask,
            }
        )
    return in_maps


def gather_output(results):
    acc = np.zeros((BS, D), dtype=np.float32)
    for r in results:
        acc += r["y_partial"].astype(np.float32)
    return acc.reshape(B, S, D)


def kernel(q, k, v, mask, Wq, Wk, Wv, Wo):
    nc = get_nc()
    in_maps = prep_inputs(q, k, v, mask, Wq, Wk, Wv, Wo)
    res = run_bass_kernel_spmd(nc, in_maps, core_ids=list(range(N_CORES)))
    return gather_output(res.results)
